# revision 1
# baseline (speedup 1.0000x reference)
"""Trainium2 Bass kernel for multi-head causal attention with RoPE.

Problem: x[4,2048,1024] -> MHA(16 heads, head_dim 64, RoPE, causal) -> [4,2048,1024]

Sharding: 8 cores = 4 batches x 2 head-groups (8 heads each, Megatron-style).
Each core computes a partial [T, C] projection output for its batch; the host
sums the two head-group partials per batch and adds b_proj.

Per-core dataflow, chunked by 512-row t-blocks so projection/attention/output
DMA all overlap (chunk qc only attends to k-chunks <= qc, so QKV for chunk qc
is ready exactly when attention chunk qc needs it):
  A(tcn): x^T via PE transposes (bf16), Q^T/K^T in [c', t] layout with RoPE
          fused on the PSUM->SBUF path, V in [t, c'] bf16 with a ones column
          (even heads [V|1], odd heads [1|V])
  B(qc=tcn): per head-pair g: scores S^T = K Q^T (bf16), block-causal with
          partial-width diagonal blocks; exp on ACT (no max subtraction,
          scores O(+-6)); P@V with the 65-col [V|1] stationary so the softmax
          denominator accumulates as a 65th PSUM row for free; denominator
          broadcast via a rank-1 PE matmul; normalize on DVE writing y^T bf16
  C(tcn): y^T @ W_proj (bf16) for this chunk, bf16 partial out DMA'd to HBM

Weights are pre-cast to bf16 and pre-laid-out on the host so every DMA moves
>=2KB contiguous runs at full modeled bandwidth; wqk is j-chunked so the first
QKV matmul can start ~2us in.
"""

import math
import sys

import numpy as np

if "/opt/trn_rl_repo" not in sys.path:
    sys.path.insert(0, "/opt/trn_rl_repo")

import concourse.bass as bass
import concourse.tile as tile
from concourse import bacc
from concourse import mybir
from concourse.bass_utils import run_bass_kernel_spmd
from concourse.masks import make_identity

B, T, C = 4, 2048, 1024
NH, D = 16, 64
HL = 8              # local heads per core
DL = HL * D         # 512
NCORES = 8
P = 128
TCH = 512           # t-chunk width
NTC = T // TCH
ROPE_BASE = 10000.0

F32 = mybir.dt.float32
F32R = mybir.dt.float32r
F16 = mybir.dt.float16
BF16 = mybir.dt.bfloat16
EXP_SHIFT = -6.25   # exp(s + EXP_SHIFT): cancels in softmax, keeps 1/denom
                    # within fp16 normal range for the broadcast matmul
Exp = mybir.ActivationFunctionType.Exp


def _emit(tc, xb, wqk, wv, wp, cos2, sin2, bias, mask, perm, out):
    nc = tc.nc
    with tc.tile_pool(name="pers", bufs=1) as pers:
        qkT = pers.tile([P, 8, T], BF16)          # j 0-3: Q pairs, 4-7: K pairs
        # V per head pair, both stationaries padded to M=128 (dst must span a
        # full legal partition range): even head [V(64)|1|0(63)] puts its
        # softmax denominator at PSUM row 64; odd head [1|0(63)|V(64)] puts
        # its denominator at row 0 and V at rows 64..127, partition-aligned
        # with yT's odd-head half. Pad columns are memset once.
        vsb = pers.tile([P, 16, 4 * 256], BF16)   # [t mod 128, t tile, pair*256+e]
        ident = pers.tile([P, P], BF16)
        make_identity(nc, ident)
        # selector matrices for the denominator broadcast: a full-K=128
        # matmul sel^T @ rcp replicates rcp row 64 (sel0) / row 0 (sel1)
        # across 64 output partitions; other rcp rows hit zeros.
        ebias = pers.tile([P, 1], F32)
        nc.vector.memset(ebias[:], EXP_SHIFT)
        sel0 = pers.tile([P, 64], F16)
        nc.vector.memset(sel0[:], 0.0)
        nc.vector.memset(sel0[64:65, :], 1.0)
        sel1 = pers.tile([P, 64], F16)
        nc.vector.memset(sel1[:], 0.0)
        nc.vector.memset(sel1[0:1, :], 1.0)

        wqk_sb = pers.tile([P, 8, 8, P], BF16)    # [p, j, o, n]
        wv_sb = pers.tile([P, 8, DL], BF16)
        wp_sb = pers.tile([P, 4, C], BF16)
        cos_sb = pers.tile([P, T], BF16)
        sin_sb = pers.tile([P, T], F32)
        bias_sb = pers.tile([P, 8 + DL], F32)
        mask_sb = pers.tile([P, 4, 512], BF16)
        perm_sb = pers.tile([P, P], BF16)
        # weight loads spread over the two HWDGE queues (cheap descriptor
        # gen), ordered by first use so the PE can start ~2us in; wv/mask/wp
        # are emitted later (after chunk 0's x loads) since the modeled DMA
        # resource serializes transfers in arrival order
        nc.scalar.dma_start(wqk_sb[:, 0], wqk[0])

        with tc.tile_pool(name="stage", bufs=3) as pstg, \
             tc.tile_pool(name="xT", bufs=2) as pxT, \
             tc.tile_pool(name="tmp", bufs=4) as ptmp, \
             tc.tile_pool(name="pt", bufs=6) as ppt, \
             tc.tile_pool(name="rcp", bufs=2) as prcp, \
             tc.tile_pool(name="rb", bufs=2) as prb, \
             tc.tile_pool(name="yT", bufs=2) as pyT, \
             tc.tile_pool(name="ost", bufs=3) as post, \
             tc.tile_pool(name="psA", bufs=2, space="PSUM") as psA, \
             tc.tile_pool(name="psQ", bufs=2, space="PSUM") as psQ, \
             tc.tile_pool(name="psS", bufs=2, space="PSUM") as psS:

            # pre-zero the pt ring: diagonal blocks read (then mask to zero)
            # columns their partial-width exp never wrote, so the ring must
            # start finite. Same for the rcp ring (broadcast matmuls contract
            # its unwritten rows against zeros, but they must be finite).
            for _ in range(6):
                ptz = ppt.tile([P, 1024], BF16, tag="pt")
                nc.gpsimd.memset(ptz[:], 0.0)
            for _ in range(2):
                rcpz = prcp.tile([P, 512], F16, tag="rcp")
                nc.gpsimd.memset(rcpz[:], 0.0)

            bias_v = bias_sb[:, 8:8 + DL].rearrange(
                "p (two pr e) -> p two pr e", two=2, e=64)
            bias_ve = bias_v[:, 0]
            bias_vo = bias_v[:, 1]
            vgv = vsb.rearrange("p a (pr e) -> p a pr e", e=256)
            nc.gpsimd.memset(vgv[:, :, :, 64:65], 1.0)
            nc.gpsimd.memset(vgv[:, :, :, 65:128], 0.0)
            nc.gpsimd.memset(vgv[:, :, :, 128:129], 1.0)
            nc.gpsimd.memset(vgv[:, :, :, 129:192], 0.0)

            def a_units(tcn):
                """Emitter units for chunk tcn's QKV phase (transposes, QK
                GEMM+RoPE, V GEMM), interleavable into the previous chunk's
                attention phase."""
                ts0 = tcn * TCH
                xT = pxT.tile([P, 8, TCH], BF16, tag="xT")
                units = []

                def tr_unit(i, xT=xT, ts0=ts0, tcn=tcn):
                    stg = pstg.tile([P, C], BF16, tag="stg")
                    nc.sync.dma_start(stg[:],
                                      xb[ts0 + i * P: ts0 + (i + 1) * P, :])
                    for quad in range(2):
                        pst = psA.tile([P, 512], BF16, tag="a")
                        for q in range(4):
                            cc = quad * 4 + q
                            nc.tensor.matmul(
                                pst[:, q * P:(q + 1) * P],
                                stg[:, cc * P:(cc + 1) * P], ident[:],
                                is_transpose=True, skip_group_check=True)
                        dstx = xT[:, quad * 4:(quad + 1) * 4, i * P:(i + 1) * P]
                        src = pst.rearrange("p (a b) -> p a b", b=P)
                        if quad % 2 and tcn > 0:
                            nc.scalar.copy(dstx, src)
                        else:
                            nc.vector.tensor_copy(dstx, src)
                for i in range(TCH // P):
                    units.append((False, lambda i=i: tr_unit(i)))

                rope_st = {"prev": None}

                def qk_unit(j, last, xT=xT, ts0=ts0):
                    psq = psQ.tile([P, TCH], F32, tag="q")
                    for cc in range(8):
                        nc.tensor.matmul(
                            psq[:],
                            wqk_sb[:, j, cc, :],
                            xT[:, cc, :],
                            start=(cc == 0), stop=(cc == 7))
                    t1 = ptmp.tile([P, TCH], BF16, tag="t1")
                    nc.vector.tensor_scalar_add(t1[:], psq[:],
                                                bias_sb[:, j:j + 1])
                    if rope_st["prev"] is not None:
                        rope_st["prev"]()

                    def rope_now(j=j, t1=t1):
                        psw = psA.tile([P, TCH], F32, tag="a")
                        nc.tensor.matmul(psw[:], perm_sb[:], t1[:],
                                         start=True, stop=True)
                        dst = qkT[:, j, ts0:ts0 + TCH]
                        nc.vector.tensor_mul(dst, t1[:],
                                             cos_sb[:, ts0:ts0 + TCH])
                        swp = ptmp.tile([P, TCH], BF16, tag="swp")
                        nc.vector.tensor_mul(swp[:], psw[:],
                                             sin_sb[:, ts0:ts0 + TCH])
                        nc.vector.tensor_tensor(dst, dst, swp[:],
                                                mybir.AluOpType.add)
                    rope_st["prev"] = rope_now
                    if last:
                        rope_st["prev"]()
                        rope_st["prev"] = None
                for j in range(8):
                    units.append((False, lambda j=j: qk_unit(j, j == 7)))

                def v_unit(i, xT=xT, tcn=tcn):
                    ti = tcn * (TCH // P) + i
                    psv = psQ.tile([P, DL], F32, tag="q")
                    for cc in range(8):
                        nc.tensor.matmul(
                            psv[:],
                            xT[:, cc, i * P:(i + 1) * P],
                            wv_sb[:, cc, :],
                            start=(cc == 0), stop=(cc == 7))
                    vv = vsb[:, ti].rearrange("p (pr e) -> p pr e", e=256)
                    psvh = psv.rearrange(
                        "p (two pr e) -> p two pr e", two=2, e=64)
                    nc.vector.tensor_tensor(
                        vv[:, :, 0:64], psvh[:, 0],
                        bias_ve, mybir.AluOpType.add)
                    nc.vector.tensor_tensor(
                        vv[:, :, 192:256], psvh[:, 1],
                        bias_vo, mybir.AluOpType.add)
                for i in range(TCH // P):
                    units.append((True, lambda i=i: v_unit(i)))
                return units

            def c_units(tcn, yT):
                """Projection units for chunk tcn; require yT fully normalized.
                Emitted inside the NEXT chunk's attention phase."""
                ts0 = tcn * TCH
                units = []

                def c_unit(i, n, yT=yT, ts0=ts0):
                    psp = psQ.tile([P, 512], F32, tag="q")
                    for g in range(4):
                        nc.tensor.matmul(
                            psp[:],
                            yT[:, g, i * P:(i + 1) * P],
                            wp_sb[:, g, n * 512:(n + 1) * 512],
                            start=(g == 0), stop=(g == 3))
                    ost = post.tile([P, 512], BF16, tag="ost")
                    nc.scalar.copy(ost[:], psp[:])
                    nc.sync.dma_start(
                        out[ts0 + i * P: ts0 + (i + 1) * P,
                            n * 512:(n + 1) * 512], ost[:])
                for i in range(TCH // P):
                    for n in range(2):
                        units.append((True, lambda i=i, n=n: c_unit(i, n)))
                return units

            def b_phase(qc, feed):
                """Attention for chunk qc. `feed` holds interleavable units
                (next chunk's A, previous chunk's C) emitted at head-pair
                boundaries so the PE chews on them while the ACT exps.
                PV matmuls lag their scores by two blocks."""
                nfull = 4 * qc
                yT = pyT.tile([P, 4, TCH], BF16, tag="yT")
                for g in range(4):
                    psO = []  # allocated lazily at the first PV emission so
                    # feed units can use the psA ring during blocks 0-1
                    pv_q = []
                    for kc in range(nfull + 4):
                        if kc in (0, 1):
                            budget = 2 if nfull == 0 else 1
                            while budget and feed:
                                feed.pop(0)[1]()
                                budget -= 1
                        m = kc - nfull  # >= 0: diagonal block band
                        pss = psS.tile([P, 1024], F32, tag="pss")
                        pt = ppt.tile([P, 1024], BF16, tag="pt")
                        for hh in range(2):
                            pb0 = hh * 64
                            q0 = m * P if m > 0 else 0
                            nc.tensor.matmul(
                                pss[:, hh * 512 + q0:(hh + 1) * 512],
                                qkT[pb0:pb0 + 64, 4 + g, kc * P:(kc + 1) * P],
                                qkT[pb0:pb0 + 64, g,
                                    qc * 512 + q0:(qc + 1) * 512],
                                start=True, stop=True)
                        if m < 0:
                            nc.scalar.activation(pt[:], pss[:], Exp,
                                                 bias=ebias[:])
                        else:
                            q0 = m * P if m > 0 else 0
                            for hh in range(2):
                                nc.scalar.activation(
                                    pt[:, hh * 512 + q0:(hh + 1) * 512],
                                    pss[:, hh * 512 + q0:(hh + 1) * 512], Exp,
                                    bias=ebias[:])
                            ptv = pt.rearrange("p (a b) -> p a b", b=512)
                            nc.vector.tensor_mul(
                                ptv, ptv,
                                mask_sb[:, m:m + 1, :].to_broadcast(
                                    (P, 2, 512)))
                        if len(pv_q) >= 2:
                            pv_q.pop(0)()

                        def pv_now(kc=kc, pt=pt):
                            if not psO:
                                psO.append(psA.tile([P, 512], F32, tag="a",
                                                    name="psO0"))
                                psO.append(psA.tile([P, 512], F32, tag="a",
                                                    name="psO1"))
                            psO0, psO1 = psO
                            nc.tensor.matmul(
                                psO0[:],
                                vgv[:, kc, g][:, 0:128],
                                pt[:, 0:512],
                                start=(kc == 0), stop=(kc == nfull + 3),
                                skip_group_check=True)
                            nc.tensor.matmul(
                                psO1[:],
                                vgv[:, kc, g][:, 128:256],
                                pt[:, 512:1024],
                                start=(kc == 0), stop=(kc == nfull + 3),
                                skip_group_check=True)
                        pv_q.append(pv_now)
                    for f in pv_q:
                        f()
                    psO0, psO1 = psO
                    rcp = prcp.tile([P, 512], F16, tag="rcp")
                    with nc.allow_low_precision(
                            reason="1/denom fits fp16 after EXP_SHIFT"):
                        nc.vector.reciprocal(rcp[64:65, :], psO0[64:65, :])
                        nc.vector.reciprocal(rcp[0:1, :], psO1[0:1, :])
                    # one interleaved unit covers the reciprocal latency.
                    # Only tag-a-free units are legal here: psO is still live,
                    # so a unit allocating from the psA ring would block the
                    # PE ahead of the psB matmuls that free it.
                    if feed and feed[0][0]:
                        feed.pop(0)[1]()
                    psB = psQ.tile([P, 512], F32, tag="q")
                    nc.tensor.matmul(psB[0:64, :], sel0[:], rcp[:],
                                     start=True, stop=True,
                                     skip_group_check=True)
                    nc.tensor.matmul(psB[64:128, :], sel1[:], rcp[:],
                                     start=True, stop=True,
                                     skip_group_check=True)
                    rb = prb.tile([P, 512], F32, tag="rb")
                    nc.vector.tensor_copy(rb[:], psB[:])
                    nc.vector.tensor_mul(yT[0:64, g, :], psO0[0:64, :],
                                         rb[0:64, :])
                    nc.vector.tensor_mul(yT[64:128, g, :], psO1[64:128, :],
                                         rb[64:128, :])
                    # drain a share of the feed at this head-pair boundary
                    share = (len(feed) + 3 - g) // (4 - g) if g < 3 else 0
                    for _ in range(share):
                        feed.pop(0)[1]()
                # whatever's left runs after the last normalize
                while feed:
                    feed.pop(0)[1]()
                return yT

            a0 = a_units(0)
            for _, u in a0[:4]:
                u()
            for j in range(1, 4):
                nc.scalar.dma_start(wqk_sb[:, j], wqk[j])
            nc.scalar.dma_start(bias_sb[:], bias)
            nc.scalar.dma_start(cos_sb[:], cos2)
            nc.scalar.dma_start(sin_sb[:], sin2)
            nc.scalar.dma_start(perm_sb[:], perm)
            for j in range(4, 8):
                nc.scalar.dma_start(wqk_sb[:, j], wqk[j])
            nc.sync.dma_start(wv_sb[:], wv)
            nc.sync.dma_start(mask_sb[:], mask)
            nc.sync.dma_start(wp_sb[:], wp)
            for _, u in a0[4:]:
                u()
            yT_prev = None
            for tcn in range(NTC):
                feed = []
                if yT_prev is not None:
                    feed.extend(c_units(tcn - 1, yT_prev))
                if tcn + 1 < NTC:
                    feed.extend(a_units(tcn + 1))
                yT_prev = b_phase(tcn, feed)
            for _, u in c_units(NTC - 1, yT_prev):
                u()


def build_nc():
    nc = bacc.Bacc("TRN2", target_bir_lowering=False, debug=False)
    xb = nc.dram_tensor("xb", [T, C], BF16, kind="ExternalInput").ap()
    wqk = nc.dram_tensor("wqk", [8, P, 8, P], BF16, kind="ExternalInput").ap()
    wv = nc.dram_tensor("wv", [P, 8, DL], BF16, kind="ExternalInput").ap()
    wp = nc.dram_tensor("wp", [P, 4, C], BF16, kind="ExternalInput").ap()
    cos2 = nc.dram_tensor("cos2", [P, T], BF16, kind="ExternalInput").ap()
    sin2 = nc.dram_tensor("sin2", [P, T], F32, kind="ExternalInput").ap()
    bias = nc.dram_tensor("bias", [P, 8 + DL], F32, kind="ExternalInput").ap()
    mask = nc.dram_tensor("mask", [P, 4, 512], BF16, kind="ExternalInput").ap()
    perm = nc.dram_tensor("perm", [P, P], BF16, kind="ExternalInput").ap()
    out = nc.dram_tensor("out", [T, C], BF16, kind="ExternalOutput").ap()
    with tile.TileContext(nc) as tc:
        _emit(tc, xb, wqk, wv, wp, cos2, sin2, bias, mask, perm, out)
    nc.compile()
    return nc


def rope_tables():
    inv_freq = 1.0 / (ROPE_BASE ** (np.arange(0, D, 2, dtype=np.float64) / D))
    t = np.arange(T, dtype=np.float64)
    freqs = np.outer(t, inv_freq)                      # [T, 32]
    emb = np.concatenate([freqs, freqs], axis=-1)      # [T, 64]
    cosT = np.cos(emb).T.astype(np.float32)            # [64, T]
    sinT = np.sin(emb).T.astype(np.float32)
    cos2 = np.tile(cosT, (2, 1)).copy()                # [128, T]
    sin2 = np.tile(sinT, (2, 1)).copy()
    return cos2, sin2


def perm_matrix():
    pm = np.zeros((P, P), dtype=np.float32)
    for base in (0, 64):
        for d in range(32):
            pm[base + d + 32, base + d] = -1.0       # rot_half: -x2 into top
            pm[base + d, base + d + 32] = 1.0        # +x1 into bottom
    return pm


def causal_masks():
    k = np.arange(P)[:, None]
    q = np.arange(512)[None, :]
    import ml_dtypes
    m = np.stack([(mm * P + k <= q) for mm in range(4)], axis=1)
    return np.ascontiguousarray(m.astype(ml_dtypes.bfloat16))  # [128, 4, 512]


def host_inputs(x, W_qkv, b_qkv, W_proj, b_proj):
    import ml_dtypes
    bf16 = ml_dtypes.bfloat16
    x = np.asarray(x, dtype=np.float32)
    W_qkv = np.asarray(W_qkv, dtype=np.float32)
    b_qkv = np.asarray(b_qkv, dtype=np.float32)
    W_proj = np.asarray(W_proj, dtype=np.float32)
    scale = 1.0 / math.sqrt(D)
    cos2, sin2 = rope_tables()
    cos2 = cos2.astype(bf16)
    masks = causal_masks()
    pm = perm_matrix().astype(bf16)
    in_maps = []
    for core in range(NCORES):
        b = core // 2
        hg = core % 2
        s = hg * DL
        wq = W_qkv[:, s:s + DL] * scale
        wk = W_qkv[:, C + s:C + s + DL]
        wqk_f = np.concatenate([wq, wk], axis=1)                # [1024, 1024]
        # [o*128+p, j*128+n] -> [j, p, o, n]
        wqk_d = np.ascontiguousarray(
            wqk_f.reshape(8, P, 8, P).transpose(2, 1, 0, 3).astype(bf16))
        ord_eo = [0, 2, 4, 6, 1, 3, 5, 7]
        wv_f = W_qkv[:, 2 * C + s:2 * C + s + DL]               # [1024, 512]
        wv_f = wv_f.reshape(C, 8, 64)[:, ord_eo, :].reshape(C, DL)
        wv_d = np.ascontiguousarray(
            wv_f.reshape(8, P, DL).transpose(1, 0, 2).astype(bf16))
        wp_f = W_proj[s:s + DL, :]                              # [512, 1024]
        wp_d = np.ascontiguousarray(
            wp_f.reshape(4, P, C).transpose(1, 0, 2).astype(bf16))
        bq = b_qkv[s:s + DL] * scale
        bk = b_qkv[C + s:C + s + DL]
        bv = b_qkv[2 * C + s:2 * C + s + DL]
        bv = bv.reshape(8, 64)[ord_eo].reshape(DL)
        bqk = np.concatenate([bq, bk]).reshape(8, P).T          # [128, 8]
        bvb = np.tile(bv[None, :], (P, 1))                      # [128, 512]
        bias = np.ascontiguousarray(
            np.concatenate([bqk, bvb], axis=1).astype(np.float32))
        in_maps.append({
            "xb": np.ascontiguousarray(x[b].astype(bf16)),
            "wqk": wqk_d, "wv": wv_d, "wp": wp_d,
            "cos2": cos2, "sin2": sin2, "bias": bias, "mask": masks,
            "perm": pm,
        })
    return in_maps


_NC_CACHE = {}


def run(in_maps, **kwargs):
    if "nc" not in _NC_CACHE:
        _NC_CACHE["nc"] = build_nc()
    return run_bass_kernel_spmd(
        _NC_CACHE["nc"], in_maps, core_ids=list(range(NCORES)), **kwargs)


def kernel(x, W_qkv, b_qkv, W_proj, b_proj, **extra):
    in_maps = host_inputs(x, W_qkv, b_qkv, W_proj, b_proj)
    res = run(in_maps)
    b_proj = np.asarray(b_proj, dtype=np.float32)
    out = np.empty((B, T, C), dtype=np.float32)
    for b in range(B):
        out[b] = (res.results[2 * b]["out"].astype(np.float32)
                  + res.results[2 * b + 1]["out"].astype(np.float32) + b_proj)
    return out



# revision 9
# speedup vs baseline: 1.1278x; 1.1278x over previous
"""Trainium2 Bass kernel for multi-head causal attention with RoPE.

Problem: x[4,2048,1024] -> MHA(16 heads, head_dim 64, RoPE, causal) -> [4,2048,1024]

Sharding: 8 cores = 4 batches x 2 head-groups (8 heads each, Megatron-style).
Each core computes a partial [T, C] projection output for its batch; the host
sums the two head-group partials per batch and adds b_proj.

Per-core dataflow, chunked by 512-row t-blocks:
  A(tcn): x^T via DMA-engine xbar transposes (HBM -> SBUF, no PE/DVE cost),
          Q^T/K^T via bf16 GEMMs in a [32-row x 2 d-half] layout with RoPE
          as pure row-aligned elementwise ops (no permute matmul), quantized
          to fp8e4; V in [t, h*64+e] fp8e4 with the qkv bias fused into the
          PSUM->SBUF quantize op
  B(qc=tcn): per head-pair (g, g+4): scores S^T = K Q^T as fp8 DoubleRow
          matmuls (d-halves packed, 2x); causal masking of diagonal blocks by
          an extra -64 triangular matmul accumulated into PSUM pre-exp; exp
          on ACT straight to fp8e5 (EXP_SHIFT=0, e5m2 spans e^-11..e^10);
          PV flipped (P^T stationary, V moving) as fp8 DoubleRow over
          kc-block pairs -> y[q, e] with all 128 output partitions useful;
          softmax denominator via a ones-column matmul into a shared PSUM
          bank; normalize = one reciprocal + broadcast multiply per head
          pair; y -> y^T via SBUF xbar-transpose DMA
  C(tcn): y^T @ W_proj (bf16) for this chunk, bf16 partial out DMA'd to HBM

Weights are pre-cast/laid-out on the host; wqk is j-chunked so the first QKV
matmul can start a few us in.
"""

import math
import sys

import numpy as np

if "/opt/trn_rl_repo" not in sys.path:
    sys.path.insert(0, "/opt/trn_rl_repo")

import concourse.bass as bass
import concourse.tile as tile
from concourse import bacc
from concourse import mybir
from concourse.bass_utils import run_bass_kernel_spmd
from concourse.masks import make_identity

B, T, C = 4, 2048, 1024
NH, D = 16, 64
HL = 8              # local heads per core
DL = HL * D         # 512
NCORES = 8
P = 128
TCH = 512           # t-chunk width
NTC = T // TCH
ROPE_BASE = 10000.0

F32 = mybir.dt.float32
BF16 = mybir.dt.bfloat16
F8E4 = mybir.dt.float8e4
F8E5 = mybir.dt.float8e5
DRW = mybir.MatmulPerfMode.DoubleRow
Exp = mybir.ActivationFunctionType.Exp
Mul = mybir.AluOpType.mult
Add = mybir.AluOpType.add
Sub = mybir.AluOpType.subtract


def _emit(tc, xb, wqk, wv, wp, cs, bias, tri, out):
    nc = tc.nc
    with tc.tile_pool(name="pers", bufs=1) as pers:
        wqk_sb = pers.tile([P, 8, 8, P], BF16)    # [p, j, cc, n]
        wv_sb = pers.tile([P, 8, DL], BF16)       # [p, cc, h*64+e]
        wp_sb = pers.tile([P, 4, C], BF16)        # [e2, g, n]
        cs_sb = pers.tile([P, 2, T], BF16)        # cos/sin, row r -> freq r%32
        bias_sb = pers.tile([P, 8 + DL], F32)
        tri_sb = pers.tile([P, P], BF16)          # tri[c,k] = -64*[c<k]
        ident = pers.tile([P, P], BF16)
        make_identity(nc, ident)
        ones8 = pers.tile([P, 2, 1], F8E4)
        nc.vector.memset(ones8[:], 1.0)
        # Q^T/K^T fp8: [row r = hloc*32+dd, d-half, slot(q0,q1,k0,k1), t]
        qk8 = pers.tile([P, 2, 4, T], F8E4)
        # V fp8: [t%128, kc-pair, parity, head, e]
        vsb = pers.tile([P, 8, 2, HL, D], F8E4)
        bias_v = bias_sb[:, 8:].rearrange("p (h e) -> p h e", e=D)

        # first-use-ordered weight loads; xT chunk-0 transposes interleave
        nc.scalar.dma_start(wqk_sb[:, 0], wqk[0])
        nc.scalar.dma_start(wqk_sb[:, 1], wqk[1])

        with tc.tile_pool(name="xT", bufs=2) as pxT, \
             tc.tile_pool(name="t1", bufs=3) as pt1, \
             tc.tile_pool(name="tmp", bufs=6) as ptmp, \
             tc.tile_pool(name="pt", bufs=4) as ppt, \
             tc.tile_pool(name="y", bufs=2) as py, \
             tc.tile_pool(name="rcp", bufs=2) as prcp, \
             tc.tile_pool(name="yT", bufs=2) as pyT, \
             tc.tile_pool(name="ost", bufs=3) as post, \
             tc.tile_pool(name="psS", bufs=2, space="PSUM") as psS, \
             tc.tile_pool(name="psQ", bufs=2, space="PSUM") as psQ, \
             tc.tile_pool(name="psO", bufs=1, space="PSUM") as psO_p, \
             tc.tile_pool(name="psD", bufs=1, space="PSUM") as psD_p:

            def a_units(tcn):
                """Chunk tcn's QKV phase: xbar-transpose DMAs, QK GEMM+RoPE
                to fp8, V GEMM+quantize. Interleavable units."""
                ts0 = tcn * TCH
                xT = pxT.tile([P, 8, TCH], BF16, tag="xT")
                units = []

                def xt_unit(i, xT=xT, ts0=ts0):
                    nc.sync.dma_start_transpose(
                        xT[:, :, i * P:(i + 1) * P],
                        xb[ts0 + i * P: ts0 + (i + 1) * P, :])
                for i in range(TCH // P):
                    units.append(lambda i=i: xt_unit(i))

                rope_st = {"t1": None}

                def qk_unit(j, xT=xT, ts0=ts0):
                    psq = psQ.tile([P, TCH], F32, tag="q")
                    for cc in range(8):
                        nc.tensor.matmul(
                            psq[:],
                            wqk_sb[:, j, cc, :],
                            xT[:, cc, :],
                            start=(cc == 0), stop=(cc == 7))
                    t1 = pt1.tile([P, TCH], BF16, tag="t1")
                    nc.vector.tensor_scalar_add(t1[:], psq[:],
                                                bias_sb[:, j:j + 1])
                    if j % 2 == 0:
                        rope_st["t1"] = t1
                        return
                    # j odd: both d-halves of this (kind, slot) ready
                    t1a, t1b = rope_st["t1"], t1
                    slot = j // 2          # 0,1 -> Q slots; 2,3 -> K slots
                    ca = cs_sb[:, 0, ts0:ts0 + TCH]
                    sa = cs_sb[:, 1, ts0:ts0 + TCH]
                    tA = ptmp.tile([P, TCH], BF16, tag="tmp")
                    tB = ptmp.tile([P, TCH], BF16, tag="tmp")
                    m1 = ptmp.tile([P, TCH], BF16, tag="tmp")
                    m2 = ptmp.tile([P, TCH], BF16, tag="tmp")
                    nc.vector.tensor_tensor(tA[:], t1a[:], ca, Mul)
                    nc.vector.tensor_tensor(m1[:], t1b[:], sa, Mul)
                    nc.vector.tensor_tensor(tB[:], t1b[:], ca, Mul)
                    nc.vector.tensor_tensor(m2[:], t1a[:], sa, Mul)
                    with nc.allow_low_precision(reason="fp8 attention"):
                        nc.vector.tensor_tensor(
                            qk8[:, 0, slot, ts0:ts0 + TCH], tA[:], m1[:], Sub)
                        nc.vector.tensor_tensor(
                            qk8[:, 1, slot, ts0:ts0 + TCH], tB[:], m2[:], Add)
                for j in range(8):
                    units.append(lambda j=j: qk_unit(j))

                def v_unit(i, xT=xT, tcn=tcn):
                    ti = tcn * (TCH // P) + i
                    psv = psQ.tile([P, DL], F32, tag="q")
                    for cc in range(8):
                        nc.tensor.matmul(
                            psv[:],
                            xT[:, cc, i * P:(i + 1) * P],
                            wv_sb[:, cc, :],
                            start=(cc == 0), stop=(cc == 7))
                    vv = vsb[:, ti // 2, ti % 2]      # [128, HL, D]
                    psvh = psv.rearrange("p (h e) -> p h e", e=D)
                    with nc.allow_low_precision(reason="fp8 attention"):
                        nc.vector.tensor_tensor(vv, psvh, bias_v, Add)
                for i in range(TCH // P):
                    units.append(lambda i=i: v_unit(i))
                return units

            def c_units(tcn, yTt):
                """Projection for chunk tcn; needs yTt complete."""
                ts0 = tcn * TCH
                units = []

                def c_unit(i, n, yTt=yTt, ts0=ts0):
                    psp = psQ.tile([P, 512], F32, tag="q")
                    for g in range(4):
                        nc.tensor.matmul(
                            psp[:],
                            yTt[:, g, i * P:(i + 1) * P],
                            wp_sb[:, g, n * 512:(n + 1) * 512],
                            start=(g == 0), stop=(g == 3))
                    ost = post.tile([P, 512], BF16, tag="ost")
                    nc.vector.tensor_copy(ost[:], psp[:])
                    nc.sync.dma_start(
                        out[ts0 + i * P: ts0 + (i + 1) * P,
                            n * 512:(n + 1) * 512], ost[:])
                for i in range(TCH // P):
                    for n in range(2):
                        units.append(lambda i=i, n=n: c_unit(i, n))
                return units

            def b_phase(qc, feed):
                """Attention for q-chunk qc; drains `feed` units into PE
                slack while ACT exps."""
                nblk = 4 * qc + 4
                yTt = pyT.tile([P, 4, TCH], BF16, tag="yT")
                for g in range(4):
                    psO = psO_p.tile([P, 8, D], F32, tag="o")
                    psD = psD_p.tile([P, 8], F32, tag="d")
                    pt_pair = [None]   # current pt tile (kc parity pair)
                    pv_q = []          # (pp, pt) with 1-pair lag

                    def pv_pair(pp, pt, qc=qc, g=g, psO=psO, psD=psD):
                        for hh in range(2):
                            head = g + 4 * hh
                            for qi in range(4):
                                last_kc = 4 * qc + qi
                                if 2 * pp > last_kc:
                                    continue
                                start = (pp == 0)
                                if 2 * pp + 1 <= last_kc:
                                    stop = (2 * pp + 1 >= last_kc)
                                    lhs = pt[:, :, hh * 512 + qi * P:
                                             hh * 512 + (qi + 1) * P]
                                    nc.tensor.matmul(
                                        psO[:, hh * 4 + qi, :], lhs,
                                        vsb[:, pp, :, head, :],
                                        start=start, stop=stop,
                                        perf_mode=DRW,
                                        skip_group_check=True)
                                    nc.tensor.matmul(
                                        psD[:, hh * 4 + qi: hh * 4 + qi + 1],
                                        lhs, ones8[:],
                                        start=start, stop=stop,
                                        perf_mode=DRW,
                                        skip_group_check=True)
                                else:   # single parity-0 block
                                    lhs = pt[:, 0, hh * 512 + qi * P:
                                             hh * 512 + (qi + 1) * P]
                                    nc.tensor.matmul(
                                        psO[:, hh * 4 + qi, :], lhs,
                                        vsb[:, pp, 0, head, :],
                                        start=start, stop=True,
                                        skip_group_check=True)
                                    nc.tensor.matmul(
                                        psD[:, hh * 4 + qi: hh * 4 + qi + 1],
                                        lhs, ones8[:, 0, :],
                                        start=start, stop=True,
                                        skip_group_check=True)

                    for kc in range(nblk):
                        if kc >= 2 and feed:
                            feed.pop(0)()
                        m = kc - 4 * qc
                        par = kc % 2
                        if par == 0:
                            ptnew = ppt.tile([P, 2, 1024], F8E5, tag="pt",
                                             name=f"pt_{qc}_{g}_{kc}")
                            pt_pair[0] = ptnew
                        pt = pt_pair[0]
                        pss = psS.tile([P, 1024], F32, tag="s")
                        for hh in range(2):
                            q0 = m * P if m > 0 else 0
                            nc.tensor.matmul(
                                pss[:, hh * 512 + q0:(hh + 1) * 512],
                                qk8[32 * g:32 * g + 32, :, 2 + hh,
                                    kc * P:(kc + 1) * P],
                                qk8[32 * g:32 * g + 32, :, hh,
                                    qc * TCH + q0:(qc + 1) * TCH],
                                start=True, stop=(m < 0),
                                perf_mode=DRW, skip_group_check=True,
                                tile_position=(32 * g, 0))
                            if m >= 0:
                                nc.tensor.matmul(
                                    pss[:, hh * 512 + q0:
                                        hh * 512 + q0 + P],
                                    tri_sb[:], ident[:],
                                    start=False, stop=True,
                                    skip_group_check=True)
                        with nc.allow_low_precision(reason="fp8 attention"):
                            if m < 0:
                                nc.scalar.activation(pt[:, par, :], pss[:],
                                                     Exp)
                            else:
                                q0 = m * P
                                for hh in range(2):
                                    nc.scalar.activation(
                                        pt[:, par,
                                           hh * 512 + q0:(hh + 1) * 512],
                                        pss[:, hh * 512 + q0:(hh + 1) * 512],
                                        Exp)
                        if par == 1:
                            if pv_q:
                                pv_pair(*pv_q.pop(0))
                            pv_q.append((kc // 2, pt))
                    while pv_q:
                        pv_pair(*pv_q.pop(0))

                    rcp = prcp.tile([P, 8], F32, tag="rcp")
                    nc.vector.reciprocal(rcp[:], psD[:])
                    y = py.tile([P, 4, P], BF16, tag="y")
                    yv = y.rearrange("p a (h e) -> p h a e", h=2)
                    psOv = psO.rearrange("p (h a) e -> p h a e", h=2)
                    for hh in range(2):
                        nc.vector.tensor_tensor(
                            yv[:, hh], psOv[:, hh],
                            rcp[:, hh * 4:(hh + 1) * 4].rearrange(
                                "p (a o) -> p a o", o=1).to_broadcast(
                                    (P, 4, D)),
                            Mul)
                    nc.sync.dma_start_transpose(
                        yTt[:, g].rearrange("p (d c) -> p d c", c=P), y[:])
                    if feed and g < 3:
                        feed.pop(0)()
                while feed:
                    feed.pop(0)()
                return yTt

            a0 = a_units(0)
            for u in a0[:4]:           # x^T transposes (SP queue)
                u()
            nc.scalar.dma_start(bias_sb[:], bias)
            nc.scalar.dma_start(cs_sb[:], cs)
            for u in a0[4:6]:          # qk j=0,1
                u()
            for j in range(2, 4):
                nc.scalar.dma_start(wqk_sb[:, j], wqk[j])
            for u in a0[6:8]:          # qk j=2,3
                u()
            for j in range(4, 8):
                nc.scalar.dma_start(wqk_sb[:, j], wqk[j])
            nc.sync.dma_start(wv_sb[:], wv)
            nc.scalar.dma_start(tri_sb[:], tri)
            nc.sync.dma_start(wp_sb[:], wp)
            for u in a0[8:]:           # qk j=4..7, v units
                u()
            yT_prev = None
            for tcn in range(NTC):
                feed = []
                if yT_prev is not None:
                    feed.extend(c_units(tcn - 1, yT_prev))
                if tcn + 1 < NTC:
                    feed.extend(a_units(tcn + 1))
                yT_prev = b_phase(tcn, feed)
            for u in c_units(NTC - 1, yT_prev):
                u()


def build_nc():
    nc = bacc.Bacc("TRN2", target_bir_lowering=False, debug=False)
    xb = nc.dram_tensor("xb", [T, C], BF16, kind="ExternalInput").ap()
    wqk = nc.dram_tensor("wqk", [8, P, 8, P], BF16, kind="ExternalInput").ap()
    wv = nc.dram_tensor("wv", [P, 8, DL], BF16, kind="ExternalInput").ap()
    wp = nc.dram_tensor("wp", [P, 4, C], BF16, kind="ExternalInput").ap()
    cs = nc.dram_tensor("cs", [P, 2, T], BF16, kind="ExternalInput").ap()
    bias = nc.dram_tensor("bias", [P, 8 + DL], F32, kind="ExternalInput").ap()
    tri = nc.dram_tensor("tri", [P, P], BF16, kind="ExternalInput").ap()
    out = nc.dram_tensor("out", [T, C], BF16, kind="ExternalOutput").ap()
    with tile.TileContext(nc) as tc:
        _emit(tc, xb, wqk, wv, wp, cs, bias, tri, out)
    nc.compile()
    return nc


def rope_tables():
    inv_freq = 1.0 / (ROPE_BASE ** (np.arange(0, D, 2, dtype=np.float64) / D))
    t = np.arange(T, dtype=np.float64)
    freqs = np.outer(t, inv_freq)                      # [T, 32]
    cosT = np.cos(freqs).T.astype(np.float32)          # [32, T]
    sinT = np.sin(freqs).T.astype(np.float32)
    cos4 = np.tile(cosT, (4, 1))                       # [128, T]
    sin4 = np.tile(sinT, (4, 1))
    return np.ascontiguousarray(np.stack([cos4, sin4], axis=1))  # [128,2,T]


def host_inputs(x, W_qkv, b_qkv, W_proj, b_proj):
    import ml_dtypes
    bf16 = ml_dtypes.bfloat16
    x = np.asarray(x, dtype=np.float32)
    W_qkv = np.asarray(W_qkv, dtype=np.float32)
    b_qkv = np.asarray(b_qkv, dtype=np.float32)
    W_proj = np.asarray(W_proj, dtype=np.float32)
    scale = 1.0 / math.sqrt(D)
    cs = rope_tables().astype(bf16)
    tri = np.zeros((P, P), dtype=np.float32)
    for c_ in range(P):
        tri[c_, c_ + 1:] = -64.0
    tri = np.ascontiguousarray(tri.astype(bf16))

    # channel order within a head-group for Q/K j-slots:
    # j = kind*4 + slot*2 + dh ; row r = hloc*32 + dd ;
    # channel (within 512-wide group) = (slot*4 + hloc)*64 + dh*32 + dd
    def qk_cols(base):
        cols = np.empty((8, P), dtype=np.int64)
        for j in range(8):
            slot, dh = (j // 2) % 2, j % 2
            for r in range(P):
                hloc, dd = r // 32, r % 32
                cols[j, r] = base + (slot * 4 + hloc) * 64 + dh * 32 + dd
        return cols

    in_maps = []
    for core in range(NCORES):
        b = core // 2
        hg = core % 2
        s = hg * DL
        # wqk: [j, p, cc, n]; input channel = cc*128 + p
        qcols = qk_cols(s)
        kcols = qk_cols(C + s)
        wqk_d = np.empty((8, P, 8, P), dtype=np.float32)
        for j in range(8):
            if j < 4:
                wcols = W_qkv[:, qcols[j]] * scale       # [1024, 128]
            else:
                wcols = W_qkv[:, kcols[j - 4]]
            wqk_d[j] = wcols.reshape(8, P, P).transpose(1, 0, 2)
        wqk_d = np.ascontiguousarray(wqk_d.astype(bf16))
        wv_f = W_qkv[:, 2 * C + s:2 * C + s + DL]        # [1024, 512]
        wv_d = np.ascontiguousarray(
            wv_f.reshape(8, P, DL).transpose(1, 0, 2).astype(bf16))
        # wp rows: e2-row p of g-tile = head (g + 4*(p//64)), e = p%64
        wp_d = np.empty((P, 4, C), dtype=np.float32)
        for g in range(4):
            for p_ in range(P):
                head = g + 4 * (p_ // 64)
                wp_d[p_, g] = W_proj[s + head * 64 + (p_ % 64), :]
        wp_d = np.ascontiguousarray(wp_d.astype(bf16))
        # bias: cols 0:8 per-j qk bias rows; cols 8: v bias tiled
        bias_d = np.zeros((P, 8 + DL), dtype=np.float32)
        bq = b_qkv[qcols] * scale                        # [8?, no: [8,128]]
        bk = b_qkv[kcols]
        for j in range(8):
            bias_d[:, j] = bq[j] if j < 4 else bk[j - 4]
        bias_d[:, 8:] = np.tile(b_qkv[2 * C + s:2 * C + s + DL][None, :],
                                (P, 1))
        in_maps.append({
            "xb": np.ascontiguousarray(x[b].astype(bf16)),
            "wqk": wqk_d, "wv": wv_d, "wp": wp_d,
            "cs": cs, "bias": np.ascontiguousarray(bias_d), "tri": tri,
        })
    return in_maps


_NC_CACHE = {}


def run(in_maps, **kwargs):
    if "nc" not in _NC_CACHE:
        _NC_CACHE["nc"] = build_nc()
    return run_bass_kernel_spmd(
        _NC_CACHE["nc"], in_maps, core_ids=list(range(NCORES)), **kwargs)


def kernel(x, W_qkv, b_qkv, W_proj, b_proj, **extra):
    in_maps = host_inputs(x, W_qkv, b_qkv, W_proj, b_proj)
    res = run(in_maps)
    b_proj = np.asarray(b_proj, dtype=np.float32)
    out = np.empty((B, T, C), dtype=np.float32)
    for b in range(B):
        out[b] = (res.results[2 * b]["out"].astype(np.float32)
                  + res.results[2 * b + 1]["out"].astype(np.float32) + b_proj)
    return out


# revision 55
# speedup vs baseline: 1.1289x; 1.0010x over previous
"""Trainium2 Bass kernel for multi-head causal attention with RoPE.

Problem: x[4,2048,1024] -> MHA(16 heads, head_dim 64, RoPE, causal) -> [4,2048,1024]

Sharding: 8 cores = 4 batches x 2 head-groups (8 heads each, Megatron-style).
Each core computes a partial [T, C] projection output for its batch; the host
sums the two head-group partials per batch and adds b_proj.

Per-core dataflow, chunked by 512-row t-blocks:
  A(tcn): x^T via DMA-engine xbar transposes (HBM -> SBUF, zero PE/DVE cost),
          Q^T/K^T bf16 GEMMs into a [2 heads x 64d] row layout with RoPE via
          a rot-half permutation matmul + elementwise muls; V in [t, h*64+e]
          bf16 with the qkv bias fused into the PSUM->SBUF copy
  B(qc=tcn): per head-pair (2g, 2g+1): scores S^T = K Q^T (bf16, K=64);
          causal masking of diagonal blocks by a -64 triangular matmul
          accumulated into the score PSUM pre-exp (no vector mask work);
          exp on ACT to bf16; PV flipped (P^T stationary, V moving) so all
          128 output partitions are useful y[q, e]; softmax denominator via
          a ones-column matmul; accumulation groups share a PSUM bank, so
          banks are pre-zeroed and all PV matmuls run with start=False
          (a start=True would mark the whole 2KB bank pending-zero and
          clobber sibling groups); normalize = one reciprocal + broadcast
          multiply per head pair; y -> y^T via one SBUF xbar-transpose DMA
          per chunk
  C(tcn): y^T @ W_proj (bf16), one batched out DMA per chunk

DMA instruction count is minimized (19 total): in this cost model each HWDGE
DMA serializes globally at ~3us (dge delay + transfer + sem propagation), so
per-chunk batching of the x-transpose, y-transpose and output store matters
more than transfer size. Weight loads are ordered by first use so the PE can
start ~6us in; chunk-0's slot-1 QK weights ride the b_phase(0) feed.
"""

import math
import sys

import numpy as np

if "/opt/trn_rl_repo" not in sys.path:
    sys.path.insert(0, "/opt/trn_rl_repo")

import concourse.bass as bass
import concourse.tile as tile
from concourse import bacc
from concourse import mybir
from concourse.bass_utils import run_bass_kernel_spmd
from concourse.masks import make_identity

B, T, C = 4, 2048, 1024
NH, D = 16, 64
HL = 8              # local heads per core
DL = HL * D         # 512
NCORES = 8
P = 128
TCH = 512           # t-chunk width
NTC = T // TCH
ROPE_BASE = 10000.0

F32 = mybir.dt.float32
BF16 = mybir.dt.bfloat16
Exp = mybir.ActivationFunctionType.Exp
Mul = mybir.AluOpType.mult
Add = mybir.AluOpType.add


def _emit(tc, xb, wqk, wv, wp, cs, bias, tri, perm, out, dbg=None):
    nc = tc.nc
    with tc.tile_pool(name="pers", bufs=1) as pers:
        wqk_sb = pers.tile([P, 8, 8, P], BF16)    # [p, j, cc, n]
        wv_sb = pers.tile([P, 8, DL], BF16)       # [p, cc, h*64+e]
        wp_sb = pers.tile([P, 4, C], BF16)        # [e2, g, n]
        cs_sb = pers.tile([P, 2, T], BF16)        # cos/sin, row r -> freq r%32
        bias_sb = pers.tile([P, 8 + DL], F32)
        tri_sb = pers.tile([P, P], BF16)          # tri[c,k] = -64*[c<k]
        perm_sb = pers.tile([P, P], BF16)         # rot-half permutation
        ident = pers.tile([P, P], BF16)
        make_identity(nc, ident)
        oneb = pers.tile([P, 1], BF16)
        nc.vector.memset(oneb[:], 1.0)
        # Q^T/K^T bf16: [row = h2*64 + d, j = 2g + kind (Q/K of pair g), t]
        qkb = pers.tile([P, 8, T], BF16)
        # V bf16: [t%128, t-tile, head, e]
        vsb = pers.tile([P, 16, HL, D], BF16)
        bias_v = bias_sb[:, 8:].rearrange("p (h e) -> p h e", e=D)

        # first-use-ordered weight loads; xT chunk-0 transpose interleaves
        nc.scalar.dma_start(wqk_sb[:, 0:4],
                            wqk[0:4].rearrange("j p cc n -> p j cc n"))

        with tc.tile_pool(name="xT", bufs=3) as pxT, \
             tc.tile_pool(name="t1", bufs=3) as pt1, \
             tc.tile_pool(name="tmp", bufs=4) as ptmp, \
             tc.tile_pool(name="pt", bufs=6) as ppt, \
             tc.tile_pool(name="y", bufs=2) as py, \
             tc.tile_pool(name="rcp", bufs=2) as prcp, \
             tc.tile_pool(name="yT", bufs=2) as pyT, \
             tc.tile_pool(name="ost", bufs=2) as post, \
             tc.tile_pool(name="psS", bufs=2, space="PSUM") as psS, \
             tc.tile_pool(name="psQ", bufs=2, space="PSUM") as psQ, \
             tc.tile_pool(name="psO", bufs=1, space="PSUM") as psO_p, \
             tc.tile_pool(name="psD", bufs=1, space="PSUM") as psD_p:

            def a_units(tcn):
                """Chunk tcn's QKV phase: xbar-transpose DMA, QK GEMM+RoPE,
                V GEMM. Units sized ~0.9us for fine interleaving."""
                ts0 = tcn * TCH
                xT = pxT.tile([P, 8, TCH], BF16, tag="xT")
                units = []

                def xt_unit(xT=xT, ts0=ts0):
                    nc.sync.dma_start_transpose(
                        xT[:], xb[ts0: ts0 + TCH, :])
                xt_list = [xt_unit]

                qk_st = {}

                def qk_half_a(j, xT=xT):
                    psq = psQ.tile([P, TCH], F32, tag="q")
                    qk_st[j] = psq
                    for cc in range(4):
                        nc.tensor.matmul(
                            psq[:],
                            wqk_sb[:, j, cc, :],
                            xT[:, cc, :],
                            start=(cc == 0), stop=False)

                def qk_unit(j, xT=xT, ts0=ts0):
                    psq = qk_st.pop(j)
                    for cc in range(4, 8):
                        nc.tensor.matmul(
                            psq[:],
                            wqk_sb[:, j, cc, :],
                            xT[:, cc, :],
                            start=False, stop=(cc == 7))
                    t1 = pt1.tile([P, TCH], BF16, tag="t1")
                    nc.vector.tensor_scalar_add(t1[:], psq[:],
                                                bias_sb[:, j:j + 1])
                    # psq is dead after the bias copy; reuse its bank for
                    # the rot-half permutation product (keeps psQ at one
                    # allocation per unit so the 2-buf ring never wraps
                    # onto a live tile)
                    nc.tensor.matmul(psq[:], perm_sb[:], t1[:],
                                     start=True, stop=True,
                                     skip_group_check=True)
                    dst = qkb[:, j, ts0:ts0 + TCH]
                    nc.vector.tensor_tensor(dst, t1[:],
                                            cs_sb[:, 0, ts0:ts0 + TCH], Mul)
                    swp = ptmp.tile([P, TCH], BF16, tag="tmp")
                    nc.vector.tensor_tensor(swp[:], psq[:],
                                            cs_sb[:, 1, ts0:ts0 + TCH], Mul)
                    nc.vector.tensor_tensor(dst, dst, swp[:], Add)
                for j in range(4):
                    units.append(lambda j=j: qk_half_a(j))
                    units.append(lambda j=j: qk_unit(j))

                def v_half_a(i, xT=xT):
                    psv = psQ.tile([P, DL], F32, tag="q")
                    qk_st[8 + i] = psv
                    for cc in range(4):
                        nc.tensor.matmul(
                            psv[:],
                            xT[:, cc, i * P:(i + 1) * P],
                            wv_sb[:, cc, :],
                            start=(cc == 0), stop=False)

                def v_unit(i, xT=xT, tcn=tcn):
                    ti = tcn * (TCH // P) + i
                    psv = qk_st.pop(8 + i)
                    for cc in range(4, 8):
                        nc.tensor.matmul(
                            psv[:],
                            xT[:, cc, i * P:(i + 1) * P],
                            wv_sb[:, cc, :],
                            start=False, stop=(cc == 7))
                    psvh = psv.rearrange("p (h e) -> p h e", e=D)
                    nc.vector.tensor_tensor(vsb[:, ti], psvh, bias_v, Add)
                for i in range(TCH // P):
                    units.append(lambda i=i: v_half_a(i))
                    units.append(lambda i=i: v_unit(i))
                for j in range(4, 8):
                    units.append(lambda j=j: qk_half_a(j))
                    units.append(lambda j=j: qk_unit(j))
                return xt_list, units

            def c_units(tcn, yTt):
                """Projection for chunk tcn; needs yTt complete."""
                ts0 = tcn * TCH
                units = []

                ost = post.tile([P, 4, C], BF16, tag="ost",
                                name=f"ost_{tcn}")

                def c_unit(i, n, yTt=yTt, ost=ost):
                    psp = psQ.tile([P, 512], F32, tag="q")
                    for g in range(4):
                        nc.tensor.matmul(
                            psp[:],
                            yTt[:, g, i * P:(i + 1) * P],
                            wp_sb[:, g, n * 512:(n + 1) * 512],
                            start=(g == 0), stop=(g == 3))
                    nc.vector.tensor_copy(
                        ost[:, i, n * 512:(n + 1) * 512], psp[:])

                def c_flush(ost=ost, ts0=ts0):
                    nc.sync.dma_start(
                        out[ts0: ts0 + TCH, :].rearrange(
                            "(i p) c -> p i c", p=P), ost[:])
                for i in range(TCH // P):
                    for n in range(2):
                        units.append(lambda i=i, n=n: c_unit(i, n))
                units.append(c_flush)
                return units

            def b_phase(qc, feed, pre):
                """Attention for q-chunk qc; drains `feed` units into PE
                slack while ACT exps. `pre` = next chunk's x-transpose
                (urgent, wait-free) then the previous chunk's y->yT
                transpose."""
                for f in pre:
                    f()
                nblk = 4 * qc + 4
                yTt = pyT.tile([P, 4, TCH], BF16, tag="yT")
                ych = py.tile([P, 4, 4, P], BF16, tag="y",
                              name=f"ych_{qc}")
                for g in range(4):
                    psO = psO_p.tile([P, 8, D], F32, tag="o")
                    psD = psD_p.tile([P, 8], F32, tag="d")
                    # pre-zero: 8 accumulation groups share each bank; a
                    # start=True would mark the whole 2KB bank pending-zero
                    # and clobber sibling groups, so accumulate-only.
                    nc.vector.memset(psO[:], 0.0)
                    nc.vector.memset(psD[:], 0.0)
                    pv_q = []          # (kc, pt) with 3-block lag

                    def pv_blk(kc, pt, qc=qc, g=g, psO=psO, psD=psD):
                        for hh in range(2):
                            head = 2 * g + hh
                            for qi in range(4):
                                last_kc = 4 * qc + qi
                                if kc > last_kc:
                                    continue
                                stop = (kc == last_kc)
                                lhs = pt[:, hh * 512 + qi * P:
                                         hh * 512 + (qi + 1) * P]
                                nc.tensor.matmul(
                                    psO[:, hh * 4 + qi, :], lhs,
                                    vsb[:, kc, head, :],
                                    start=False, stop=stop,
                                    skip_group_check=True)
                                nc.tensor.matmul(
                                    psD[:, hh * 4 + qi: hh * 4 + qi + 1],
                                    lhs, oneb[:],
                                    start=False, stop=stop,
                                    skip_group_check=True)

                    for kc in range(nblk):
                        if kc >= (2 if qc else 0) and feed:
                            feed.pop(0)()
                        m = kc - 4 * qc
                        pt = ppt.tile([P, 1024], BF16, tag="pt",
                                      name=f"pt_{qc}_{g}_{kc}")
                        pss = psS.tile([P, 1024], F32, tag="s")
                        for hh in range(2):
                            q0 = m * P if m > 0 else 0
                            nc.tensor.matmul(
                                pss[:, hh * 512 + q0:(hh + 1) * 512],
                                qkb[64 * hh:64 * hh + 64, 2 * g + 1,
                                    kc * P:(kc + 1) * P],
                                qkb[64 * hh:64 * hh + 64, 2 * g,
                                    qc * TCH + q0:(qc + 1) * TCH],
                                start=True, stop=(m < 0),
                                skip_group_check=True,
                                tile_position=(64 * hh, 0))
                            if m >= 0:
                                nc.tensor.matmul(
                                    pss[:, hh * 512 + q0:
                                        hh * 512 + q0 + P],
                                    tri_sb[:], ident[:],
                                    start=False, stop=True,
                                    skip_group_check=True)
                        with nc.allow_low_precision(reason="bf16 softmax"):
                            if m < 0:
                                nc.scalar.activation(pt[:], pss[:], Exp)
                            else:
                                q0 = m * P
                                for hh in range(2):
                                    nc.scalar.activation(
                                        pt[:, hh * 512 + q0:(hh + 1) * 512],
                                        pss[:, hh * 512 + q0:(hh + 1) * 512],
                                        Exp)
                        if len(pv_q) >= 3:
                            pv_blk(*pv_q.pop(0))
                        pv_q.append((kc, pt))
                    while pv_q:
                        pv_blk(*pv_q.pop(0))

                    rcp = prcp.tile([P, 8], F32, tag="rcp")
                    nc.vector.reciprocal(rcp[:], psD[:])
                    yv = ych[:, g].rearrange("p a (h e) -> p h a e", h=2)
                    psOv = psO.rearrange("p (h a) e -> p h a e", h=2)
                    for hh in range(2):
                        nc.vector.tensor_tensor(
                            yv[:, hh], psOv[:, hh],
                            rcp[:, hh * 4:(hh + 1) * 4].rearrange(
                                "p (a o) -> p a o", o=1).to_broadcast(
                                    (P, 4, D)),
                            Mul)
                    if feed and g < 3:
                        feed.pop(0)()
                while feed:
                    feed.pop(0)()

                def ytr(ych=ych, yTt=yTt):
                    nc.sync.dma_start_transpose(
                        yTt.rearrange("p g (a c) -> p (g a) c", c=P),
                        ych[:])
                return yTt, [ytr]

            xt0, a0 = a_units(0)
            nc.scalar.dma_start(bias_sb[:], bias)
            nc.scalar.dma_start(tri_sb[:], tri)
            nc.scalar.dma_start(perm_sb[:], perm)
            for u in xt0:              # x^T chunk-0 transpose (SP queue)
                u()
            nc.scalar.dma_start(cs_sb[:], cs)
            for u in a0[:8]:           # qk slot 0 (j=0..3)
                u()
            nc.sync.dma_start(wv_sb[:], wv)
            nc.scalar.dma_start(wqk_sb[:, 4:8],
                                wqk[4:8].rearrange("j p cc n -> p j cc n"))
            nc.sync.dma_start(wp_sb[:], wp)
            for u in a0[8:16]:         # v units (needed by b_phase(0) PV)
                u()
            # chunk-0 QK of head-pairs 2,3 ride as b_phase(0) feed: their
            # weights (wqk[4:8]) land late in the DMA chain and must not
            # block the first head-pairs' scores in PE program order; they
            # drain before g=2 needs them
            yT_prev, ytr_prev = None, []
            carry = a0[16:]
            for tcn in range(NTC):
                feed = list(carry)
                carry = []
                pre = []
                if yT_prev is not None:
                    feed.extend(c_units(tcn - 1, yT_prev))
                if tcn + 1 < NTC:
                    xt_n, a_n = a_units(tcn + 1)
                    pre.extend(xt_n)
                    feed.extend(a_n)
                pre.extend(ytr_prev)
                yT_prev, ytr_prev = b_phase(tcn, feed, pre)
            for f in ytr_prev:
                f()
            for u in c_units(NTC - 1, yT_prev):
                u()
            if dbg is not None:
                nc.sync.dma_start(dbg["qkb"], qkb[:])
                nc.sync.dma_start(dbg["vsb"], vsb[:])
                nc.sync.dma_start(dbg["yT3"], yT_prev[:])


def build_nc(debug=False):
    nc = bacc.Bacc("TRN2", target_bir_lowering=False, debug=False)
    xb = nc.dram_tensor("xb", [T, C], BF16, kind="ExternalInput").ap()
    wqk = nc.dram_tensor("wqk", [8, P, 8, P], BF16, kind="ExternalInput").ap()
    wv = nc.dram_tensor("wv", [P, 8, DL], BF16, kind="ExternalInput").ap()
    wp = nc.dram_tensor("wp", [P, 4, C], BF16, kind="ExternalInput").ap()
    cs = nc.dram_tensor("cs", [P, 2, T], BF16, kind="ExternalInput").ap()
    bias = nc.dram_tensor("bias", [P, 8 + DL], F32, kind="ExternalInput").ap()
    tri = nc.dram_tensor("tri", [P, P], BF16, kind="ExternalInput").ap()
    perm = nc.dram_tensor("perm", [P, P], BF16, kind="ExternalInput").ap()
    out = nc.dram_tensor("out", [T, C], BF16, kind="ExternalOutput").ap()
    dbg = None
    if debug:
        dbg = {
            "qkb": nc.dram_tensor("d_qkb", [P, 8, T], BF16,
                                  kind="ExternalOutput").ap(),
            "vsb": nc.dram_tensor("d_vsb", [P, 16, HL, D], BF16,
                                  kind="ExternalOutput").ap(),
            "yT3": nc.dram_tensor("d_yT3", [P, 4, TCH], BF16,
                                  kind="ExternalOutput").ap(),
        }
    with tile.TileContext(nc) as tc:
        _emit(tc, xb, wqk, wv, wp, cs, bias, tri, perm, out, dbg=dbg)
    nc.compile()
    return nc


def rope_tables():
    inv_freq = 1.0 / (ROPE_BASE ** (np.arange(0, D, 2, dtype=np.float64) / D))
    t = np.arange(T, dtype=np.float64)
    freqs = np.outer(t, inv_freq)                      # [T, 32]
    cosT = np.cos(freqs).T.astype(np.float32)          # [32, T]
    sinT = np.sin(freqs).T.astype(np.float32)
    cos4 = np.tile(cosT, (4, 1))                       # [128, T]
    sin4 = np.tile(sinT, (4, 1))
    return np.ascontiguousarray(np.stack([cos4, sin4], axis=1))  # [128,2,T]


def perm_matrix():
    pm = np.zeros((P, P), dtype=np.float32)
    for base in (0, 64):
        for d in range(32):
            pm[base + d + 32, base + d] = -1.0       # rot_half: -x2 into top
            pm[base + d, base + d + 32] = 1.0        # +x1 into bottom
    return pm


def host_inputs(x, W_qkv, b_qkv, W_proj, b_proj):
    import ml_dtypes
    bf16 = ml_dtypes.bfloat16
    x = np.asarray(x, dtype=np.float32)
    W_qkv = np.asarray(W_qkv, dtype=np.float32)
    b_qkv = np.asarray(b_qkv, dtype=np.float32)
    W_proj = np.asarray(W_proj, dtype=np.float32)
    scale = 1.0 / math.sqrt(D)
    cs = rope_tables().astype(bf16)
    tri = np.zeros((P, P), dtype=np.float32)
    for c_ in range(P):
        tri[c_, c_ + 1:] = -64.0
    tri = np.ascontiguousarray(tri.astype(bf16))
    pm = np.ascontiguousarray(perm_matrix().astype(bf16))

    in_maps = []
    for core in range(NCORES):
        b = core // 2
        hg = core % 2
        s = hg * DL
        # wqk: [j, p, cc, n]; j = 2g + kind; psq row n = h2*64 + d;
        # W col = kind*C + s + (2g + h2)*64 + d ; input channel = cc*128+p
        cols = np.empty((8, P), dtype=np.int64)
        for j in range(8):
            g, kind = j // 2, j % 2
            for n in range(P):
                h2, d = n // 64, n % 64
                cols[j, n] = kind * C + s + (2 * g + h2) * 64 + d
        wqk_d = np.empty((8, P, 8, P), dtype=np.float32)
        for j in range(8):
            wcols = W_qkv[:, cols[j]]                    # [1024, 128]
            if j % 2 == 0:                               # Q: fold 1/sqrt(D)
                wcols = wcols * scale
            wqk_d[j] = wcols.reshape(8, P, P).transpose(1, 0, 2)
        wqk_d = np.ascontiguousarray(wqk_d.astype(bf16))
        wv_f = W_qkv[:, 2 * C + s:2 * C + s + DL]        # [1024, 512]
        wv_d = np.ascontiguousarray(
            wv_f.reshape(8, P, DL).transpose(1, 0, 2).astype(bf16))
        # wp rows: e2-row p of g-tile = head 2g + p//64, e = p%64
        wp_d = np.empty((P, 4, C), dtype=np.float32)
        for g in range(4):
            for p_ in range(P):
                head = 2 * g + (p_ // 64)
                wp_d[p_, g] = W_proj[s + head * 64 + (p_ % 64), :]
        wp_d = np.ascontiguousarray(wp_d.astype(bf16))
        bias_d = np.zeros((P, 8 + DL), dtype=np.float32)
        for j in range(8):
            bias_d[:, j] = b_qkv[cols[j]]
            if j % 2 == 0:
                bias_d[:, j] *= scale
        bias_d[:, 8:] = np.tile(b_qkv[2 * C + s:2 * C + s + DL][None, :],
                                (P, 1))
        in_maps.append({
            "xb": np.ascontiguousarray(x[b].astype(bf16)),
            "wqk": wqk_d, "wv": wv_d, "wp": wp_d,
            "cs": cs, "bias": np.ascontiguousarray(bias_d), "tri": tri,
            "perm": pm,
        })
    return in_maps


_NC_CACHE = {}


def run(in_maps, **kwargs):
    if "nc" not in _NC_CACHE:
        _NC_CACHE["nc"] = build_nc()
    return run_bass_kernel_spmd(
        _NC_CACHE["nc"], in_maps, core_ids=list(range(NCORES)), **kwargs)


def kernel(x, W_qkv, b_qkv, W_proj, b_proj, **extra):
    in_maps = host_inputs(x, W_qkv, b_qkv, W_proj, b_proj)
    res = run(in_maps)
    b_proj = np.asarray(b_proj, dtype=np.float32)
    out = np.empty((B, T, C), dtype=np.float32)
    for b in range(B):
        out[b] = (res.results[2 * b]["out"].astype(np.float32)
                  + res.results[2 * b + 1]["out"].astype(np.float32) + b_proj)
    return out


# revision 56
# speedup vs baseline: 1.1318x; 1.0025x over previous
"""Trainium2 Bass kernel for multi-head causal attention with RoPE.

Problem: x[4,2048,1024] -> MHA(16 heads, head_dim 64, RoPE, causal) -> [4,2048,1024]

Sharding: 8 cores = 4 batches x 2 head-groups (8 heads each, Megatron-style).
Each core computes a partial [T, C] projection output for its batch; the host
sums the two head-group partials per batch and adds b_proj.

Per-core dataflow, chunked by 512-row t-blocks:
  A(tcn): x^T via DMA-engine xbar transposes (HBM -> SBUF, zero PE/DVE cost),
          Q^T/K^T bf16 GEMMs into a [2 heads x 64d] row layout with RoPE via
          a rot-half permutation matmul + elementwise muls; V in [t, h*64+e]
          bf16 with the qkv bias fused into the PSUM->SBUF copy
  B(qc=tcn): per head-pair (2g, 2g+1): scores S^T = K Q^T (bf16, K=64);
          causal masking of diagonal blocks by a -64 triangular matmul
          accumulated into the score PSUM pre-exp (no vector mask work);
          exp on ACT to bf16; PV flipped (P^T stationary, V moving) so all
          128 output partitions are useful y[q, e]; softmax denominator via
          a ones-column matmul; accumulation groups share a PSUM bank, so
          banks are pre-zeroed and all PV matmuls run with start=False
          (a start=True would mark the whole 2KB bank pending-zero and
          clobber sibling groups); normalize = one reciprocal + broadcast
          multiply per head pair; y -> y^T via one SBUF xbar-transpose DMA
          per chunk
  C(tcn): y^T @ W_proj (bf16), one batched out DMA per chunk

DMA instruction count is minimized (19 total): in this cost model each HWDGE
DMA serializes globally at ~3us (dge delay + transfer + sem propagation), so
per-chunk batching of the x-transpose, y-transpose and output store matters
more than transfer size. Weight loads are ordered by first use so the PE can
start ~6us in; chunk-0's slot-1 QK weights ride the b_phase(0) feed.
"""

import math
import sys

import numpy as np

if "/opt/trn_rl_repo" not in sys.path:
    sys.path.insert(0, "/opt/trn_rl_repo")

import concourse.bass as bass
import concourse.tile as tile
from concourse import bacc
from concourse import mybir
from concourse.bass_utils import run_bass_kernel_spmd
from concourse.masks import make_identity

B, T, C = 4, 2048, 1024
NH, D = 16, 64
HL = 8              # local heads per core
DL = HL * D         # 512
NCORES = 8
P = 128
TCH = 512           # t-chunk width
NTC = T // TCH
ROPE_BASE = 10000.0

F32 = mybir.dt.float32
BF16 = mybir.dt.bfloat16
Exp = mybir.ActivationFunctionType.Exp
Mul = mybir.AluOpType.mult
Add = mybir.AluOpType.add


def _emit(tc, xb, wqk, wv, wp, cs, bias, tri, perm, out, dbg=None):
    nc = tc.nc
    with tc.tile_pool(name="pers", bufs=1) as pers:
        wqk_sb = pers.tile([P, 8, 8, P], BF16)    # [p, j, cc, n]
        wv_sb = pers.tile([P, 8, DL], BF16)       # [p, cc, h*64+e]
        wp_sb = pers.tile([P, 4, C], BF16)        # [e2, g, n]
        cs_sb = pers.tile([P, 2, T], BF16)        # cos/sin, row r -> freq r%32
        bias_sb = pers.tile([P, 8 + DL], F32)
        tri_sb = pers.tile([P, P], BF16)          # tri[c,k] = -64*[c<k]
        perm_sb = pers.tile([P, P], BF16)         # rot-half permutation
        ident = pers.tile([P, P], BF16)
        make_identity(nc, ident)
        oneb = pers.tile([P, 1], BF16)
        nc.vector.memset(oneb[:], 1.0)
        # Q^T/K^T bf16: [row = h2*64 + d, j = 2g + kind (Q/K of pair g), t]
        qkb = pers.tile([P, 8, T], BF16)
        # V bf16: [t%128, t-tile, head, e | ones]; col 64 feeds the
        # softmax denominator through the same PV matmul
        vsb = pers.tile([P, 16, HL, D + 1], BF16)
        nc.vector.memset(vsb.rearrange("p a b c -> p (a b) c")[:, :, D:], 1.0)
        bias_v = bias_sb[:, 8:].rearrange("p (h e) -> p h e", e=D)

        # first-use-ordered weight loads; xT chunk-0 transpose interleaves
        nc.scalar.dma_start(wqk_sb[:, 0:4],
                            wqk[0:4].rearrange("j p cc n -> p j cc n"))

        with tc.tile_pool(name="xT", bufs=3) as pxT, \
             tc.tile_pool(name="t1", bufs=3) as pt1, \
             tc.tile_pool(name="tmp", bufs=4) as ptmp, \
             tc.tile_pool(name="pt", bufs=6) as ppt, \
             tc.tile_pool(name="y", bufs=2) as py, \
             tc.tile_pool(name="rcp", bufs=2) as prcp, \
             tc.tile_pool(name="yT", bufs=2) as pyT, \
             tc.tile_pool(name="ost", bufs=2) as post, \
             tc.tile_pool(name="psS", bufs=2, space="PSUM") as psS, \
             tc.tile_pool(name="psQ", bufs=2, space="PSUM") as psQ, \
             tc.tile_pool(name="psO", bufs=2, space="PSUM") as psO_p:

            def a_units(tcn):
                """Chunk tcn's QKV phase: xbar-transpose DMA, QK GEMM+RoPE,
                V GEMM. Units sized ~0.9us for fine interleaving."""
                ts0 = tcn * TCH
                xT = pxT.tile([P, 8, TCH], BF16, tag="xT")
                units = []

                def xt_unit(xT=xT, ts0=ts0):
                    nc.sync.dma_start_transpose(
                        xT[:], xb[ts0: ts0 + TCH, :])
                xt_list = [xt_unit]

                qk_st = {}

                def qk_half_a(j, xT=xT):
                    psq = psQ.tile([P, TCH], F32, tag="q")
                    qk_st[j] = psq
                    for cc in range(4):
                        nc.tensor.matmul(
                            psq[:],
                            wqk_sb[:, j, cc, :],
                            xT[:, cc, :],
                            start=(cc == 0), stop=False)

                def qk_unit(j, xT=xT, ts0=ts0):
                    psq = qk_st.pop(j)
                    for cc in range(4, 8):
                        nc.tensor.matmul(
                            psq[:],
                            wqk_sb[:, j, cc, :],
                            xT[:, cc, :],
                            start=False, stop=(cc == 7))
                    t1 = pt1.tile([P, TCH], BF16, tag="t1")
                    nc.vector.tensor_scalar_add(t1[:], psq[:],
                                                bias_sb[:, j:j + 1])
                    # psq is dead after the bias copy; reuse its bank for
                    # the rot-half permutation product (keeps psQ at one
                    # allocation per unit so the 2-buf ring never wraps
                    # onto a live tile)
                    nc.tensor.matmul(psq[:], perm_sb[:], t1[:],
                                     start=True, stop=True,
                                     skip_group_check=True)
                    dst = qkb[:, j, ts0:ts0 + TCH]
                    nc.vector.tensor_tensor(dst, t1[:],
                                            cs_sb[:, 0, ts0:ts0 + TCH], Mul)
                    swp = ptmp.tile([P, TCH], BF16, tag="tmp")
                    nc.vector.tensor_tensor(swp[:], psq[:],
                                            cs_sb[:, 1, ts0:ts0 + TCH], Mul)
                    nc.vector.tensor_tensor(dst, dst, swp[:], Add)
                for j in range(4):
                    units.append(lambda j=j: qk_half_a(j))
                    units.append(lambda j=j: qk_unit(j))

                def v_half_a(i, xT=xT):
                    psv = psQ.tile([P, DL], F32, tag="q")
                    qk_st[8 + i] = psv
                    for cc in range(4):
                        nc.tensor.matmul(
                            psv[:],
                            xT[:, cc, i * P:(i + 1) * P],
                            wv_sb[:, cc, :],
                            start=(cc == 0), stop=False)

                def v_unit(i, xT=xT, tcn=tcn):
                    ti = tcn * (TCH // P) + i
                    psv = qk_st.pop(8 + i)
                    for cc in range(4, 8):
                        nc.tensor.matmul(
                            psv[:],
                            xT[:, cc, i * P:(i + 1) * P],
                            wv_sb[:, cc, :],
                            start=False, stop=(cc == 7))
                    psvh = psv.rearrange("p (h e) -> p h e", e=D)
                    nc.vector.tensor_tensor(vsb[:, ti, :, 0:D], psvh,
                                            bias_v, Add)
                for i in range(TCH // P):
                    units.append(lambda i=i: v_half_a(i))
                    units.append(lambda i=i: v_unit(i))
                for j in range(4, 8):
                    units.append(lambda j=j: qk_half_a(j))
                    units.append(lambda j=j: qk_unit(j))
                return xt_list, units

            def c_units(tcn, yTt):
                """Projection for chunk tcn; needs yTt complete."""
                ts0 = tcn * TCH
                units = []

                ost = post.tile([P, 4, C], BF16, tag="ost",
                                name=f"ost_{tcn}")

                def c_unit(i, n, yTt=yTt, ost=ost):
                    psp = psQ.tile([P, 512], F32, tag="q")
                    for g in range(4):
                        nc.tensor.matmul(
                            psp[:],
                            yTt[:, g, i * P:(i + 1) * P],
                            wp_sb[:, g, n * 512:(n + 1) * 512],
                            start=(g == 0), stop=(g == 3))
                    nc.vector.tensor_copy(
                        ost[:, i, n * 512:(n + 1) * 512], psp[:])

                def c_flush(ost=ost, ts0=ts0):
                    nc.sync.dma_start(
                        out[ts0: ts0 + TCH, :].rearrange(
                            "(i p) c -> p i c", p=P), ost[:])
                for i in range(TCH // P):
                    for n in range(2):
                        units.append(lambda i=i, n=n: c_unit(i, n))
                units.append(c_flush)
                return units

            def b_phase(qc, feed, pre):
                """Attention for q-chunk qc; drains `feed` units into PE
                slack while ACT exps. `pre` = next chunk's x-transpose
                (urgent, wait-free) then the previous chunk's y->yT
                transpose."""
                for f in pre:
                    f()
                nblk = 4 * qc + 4
                yTt = pyT.tile([P, 4, TCH], BF16, tag="yT")
                ych = py.tile([P, 4, 4, P], BF16, tag="y",
                              name=f"ych_{qc}")
                for g in range(4):
                    psO0 = psO_p.tile([P, 4, D + 1], F32, tag="o",
                                      name=f"psO0_{qc}_{g}")
                    psO1 = psO_p.tile([P, 4, D + 1], F32, tag="o",
                                      name=f"psO1_{qc}_{g}")
                    psO = (psO0, psO1)
                    # pre-zero: 4 accumulation groups share each bank; a
                    # start=True would mark the whole 2KB bank pending-zero
                    # and clobber sibling groups, so accumulate-only.
                    nc.vector.memset(psO0[:], 0.0)
                    nc.vector.memset(psO1[:], 0.0)
                    pv_q = []          # (kc, pt) with 3-block lag

                    def pv_blk(kc, pt, qc=qc, g=g, psO=psO):
                        for hh in range(2):
                            head = 2 * g + hh
                            for qi in range(4):
                                last_kc = 4 * qc + qi
                                if kc > last_kc:
                                    continue
                                stop = (kc == last_kc)
                                lhs = pt[:, hh * 512 + qi * P:
                                         hh * 512 + (qi + 1) * P]
                                nc.tensor.matmul(
                                    psO[hh][:, qi, :], lhs,
                                    vsb[:, kc, head, :],
                                    start=False, stop=stop,
                                    skip_group_check=True)

                    for kc in range(nblk):
                        if kc >= (2 if qc else 0) and feed:
                            feed.pop(0)()
                        m = kc - 4 * qc
                        pt = ppt.tile([P, 1024], BF16, tag="pt",
                                      name=f"pt_{qc}_{g}_{kc}")
                        pss = psS.tile([P, 1024], F32, tag="s")
                        for hh in range(2):
                            q0 = m * P if m > 0 else 0
                            nc.tensor.matmul(
                                pss[:, hh * 512 + q0:(hh + 1) * 512],
                                qkb[64 * hh:64 * hh + 64, 2 * g + 1,
                                    kc * P:(kc + 1) * P],
                                qkb[64 * hh:64 * hh + 64, 2 * g,
                                    qc * TCH + q0:(qc + 1) * TCH],
                                start=True, stop=(m < 0),
                                skip_group_check=True,
                                tile_position=(64 * hh, 0))
                            if m >= 0:
                                nc.tensor.matmul(
                                    pss[:, hh * 512 + q0:
                                        hh * 512 + q0 + P],
                                    tri_sb[:], ident[:],
                                    start=False, stop=True,
                                    skip_group_check=True)
                        with nc.allow_low_precision(reason="bf16 softmax"):
                            if m < 0:
                                nc.scalar.activation(pt[:], pss[:], Exp)
                            else:
                                q0 = m * P
                                for hh in range(2):
                                    nc.scalar.activation(
                                        pt[:, hh * 512 + q0:(hh + 1) * 512],
                                        pss[:, hh * 512 + q0:(hh + 1) * 512],
                                        Exp)
                        if len(pv_q) >= 3:
                            pv_blk(*pv_q.pop(0))
                        pv_q.append((kc, pt))
                    while pv_q:
                        pv_blk(*pv_q.pop(0))

                    rcp = prcp.tile([P, 2, 4], F32, tag="rcp")
                    yv = ych[:, g].rearrange("p a (h e) -> p h a e", h=2)
                    for hh in range(2):
                        nc.vector.reciprocal(rcp[:, hh],
                                             psO[hh][:, :, D])
                        nc.vector.tensor_tensor(
                            yv[:, hh], psO[hh][:, :, 0:D],
                            rcp[:, hh].rearrange(
                                "p (a o) -> p a o", o=1).to_broadcast(
                                    (P, 4, D)),
                            Mul)
                    if feed and g < 3:
                        feed.pop(0)()
                while feed:
                    feed.pop(0)()

                def ytr(ych=ych, yTt=yTt):
                    nc.sync.dma_start_transpose(
                        yTt.rearrange("p g (a c) -> p (g a) c", c=P),
                        ych[:])
                return yTt, [ytr]

            xt0, a0 = a_units(0)
            nc.scalar.dma_start(bias_sb[:], bias)
            nc.scalar.dma_start(tri_sb[:], tri)
            nc.scalar.dma_start(perm_sb[:], perm)
            for u in xt0:              # x^T chunk-0 transpose (SP queue)
                u()
            nc.scalar.dma_start(cs_sb[:], cs)
            for u in a0[:8]:           # qk slot 0 (j=0..3)
                u()
            nc.sync.dma_start(wv_sb[:], wv)
            nc.scalar.dma_start(wqk_sb[:, 4:8],
                                wqk[4:8].rearrange("j p cc n -> p j cc n"))
            nc.sync.dma_start(wp_sb[:], wp)
            for u in a0[8:16]:         # v units (needed by b_phase(0) PV)
                u()
            # chunk-0 QK of head-pairs 2,3 ride as b_phase(0) feed: their
            # weights (wqk[4:8]) land late in the DMA chain and must not
            # block the first head-pairs' scores in PE program order; they
            # drain before g=2 needs them
            yT_prev, ytr_prev = None, []
            carry = a0[16:]
            for tcn in range(NTC):
                feed = list(carry)
                carry = []
                pre = []
                if yT_prev is not None:
                    feed.extend(c_units(tcn - 1, yT_prev))
                if tcn + 1 < NTC:
                    xt_n, a_n = a_units(tcn + 1)
                    pre.extend(xt_n)
                    feed.extend(a_n)
                pre.extend(ytr_prev)
                yT_prev, ytr_prev = b_phase(tcn, feed, pre)
            for f in ytr_prev:
                f()
            for u in c_units(NTC - 1, yT_prev):
                u()
            if dbg is not None:
                nc.sync.dma_start(dbg["qkb"], qkb[:])
                nc.sync.dma_start(dbg["vsb"], vsb[:])
                nc.sync.dma_start(dbg["yT3"], yT_prev[:])


def build_nc(debug=False):
    nc = bacc.Bacc("TRN2", target_bir_lowering=False, debug=False)
    xb = nc.dram_tensor("xb", [T, C], BF16, kind="ExternalInput").ap()
    wqk = nc.dram_tensor("wqk", [8, P, 8, P], BF16, kind="ExternalInput").ap()
    wv = nc.dram_tensor("wv", [P, 8, DL], BF16, kind="ExternalInput").ap()
    wp = nc.dram_tensor("wp", [P, 4, C], BF16, kind="ExternalInput").ap()
    cs = nc.dram_tensor("cs", [P, 2, T], BF16, kind="ExternalInput").ap()
    bias = nc.dram_tensor("bias", [P, 8 + DL], F32, kind="ExternalInput").ap()
    tri = nc.dram_tensor("tri", [P, P], BF16, kind="ExternalInput").ap()
    perm = nc.dram_tensor("perm", [P, P], BF16, kind="ExternalInput").ap()
    out = nc.dram_tensor("out", [T, C], BF16, kind="ExternalOutput").ap()
    dbg = None
    if debug:
        dbg = {
            "qkb": nc.dram_tensor("d_qkb", [P, 8, T], BF16,
                                  kind="ExternalOutput").ap(),
            "vsb": nc.dram_tensor("d_vsb", [P, 16, HL, D], BF16,
                                  kind="ExternalOutput").ap(),
            "yT3": nc.dram_tensor("d_yT3", [P, 4, TCH], BF16,
                                  kind="ExternalOutput").ap(),
        }
    with tile.TileContext(nc) as tc:
        _emit(tc, xb, wqk, wv, wp, cs, bias, tri, perm, out, dbg=dbg)
    nc.compile()
    return nc


def rope_tables():
    inv_freq = 1.0 / (ROPE_BASE ** (np.arange(0, D, 2, dtype=np.float64) / D))
    t = np.arange(T, dtype=np.float64)
    freqs = np.outer(t, inv_freq)                      # [T, 32]
    cosT = np.cos(freqs).T.astype(np.float32)          # [32, T]
    sinT = np.sin(freqs).T.astype(np.float32)
    cos4 = np.tile(cosT, (4, 1))                       # [128, T]
    sin4 = np.tile(sinT, (4, 1))
    return np.ascontiguousarray(np.stack([cos4, sin4], axis=1))  # [128,2,T]


def perm_matrix():
    pm = np.zeros((P, P), dtype=np.float32)
    for base in (0, 64):
        for d in range(32):
            pm[base + d + 32, base + d] = -1.0       # rot_half: -x2 into top
            pm[base + d, base + d + 32] = 1.0        # +x1 into bottom
    return pm


def host_inputs(x, W_qkv, b_qkv, W_proj, b_proj):
    import ml_dtypes
    bf16 = ml_dtypes.bfloat16
    x = np.asarray(x, dtype=np.float32)
    W_qkv = np.asarray(W_qkv, dtype=np.float32)
    b_qkv = np.asarray(b_qkv, dtype=np.float32)
    W_proj = np.asarray(W_proj, dtype=np.float32)
    scale = 1.0 / math.sqrt(D)
    cs = rope_tables().astype(bf16)
    tri = np.zeros((P, P), dtype=np.float32)
    for c_ in range(P):
        tri[c_, c_ + 1:] = -64.0
    tri = np.ascontiguousarray(tri.astype(bf16))
    pm = np.ascontiguousarray(perm_matrix().astype(bf16))

    in_maps = []
    for core in range(NCORES):
        b = core // 2
        hg = core % 2
        s = hg * DL
        # wqk: [j, p, cc, n]; j = 2g + kind; psq row n = h2*64 + d;
        # W col = kind*C + s + (2g + h2)*64 + d ; input channel = cc*128+p
        cols = np.empty((8, P), dtype=np.int64)
        for j in range(8):
            g, kind = j // 2, j % 2
            for n in range(P):
                h2, d = n // 64, n % 64
                cols[j, n] = kind * C + s + (2 * g + h2) * 64 + d
        wqk_d = np.empty((8, P, 8, P), dtype=np.float32)
        for j in range(8):
            wcols = W_qkv[:, cols[j]]                    # [1024, 128]
            if j % 2 == 0:                               # Q: fold 1/sqrt(D)
                wcols = wcols * scale
            wqk_d[j] = wcols.reshape(8, P, P).transpose(1, 0, 2)
        wqk_d = np.ascontiguousarray(wqk_d.astype(bf16))
        wv_f = W_qkv[:, 2 * C + s:2 * C + s + DL]        # [1024, 512]
        wv_d = np.ascontiguousarray(
            wv_f.reshape(8, P, DL).transpose(1, 0, 2).astype(bf16))
        # wp rows: e2-row p of g-tile = head 2g + p//64, e = p%64
        wp_d = np.empty((P, 4, C), dtype=np.float32)
        for g in range(4):
            for p_ in range(P):
                head = 2 * g + (p_ // 64)
                wp_d[p_, g] = W_proj[s + head * 64 + (p_ % 64), :]
        wp_d = np.ascontiguousarray(wp_d.astype(bf16))
        bias_d = np.zeros((P, 8 + DL), dtype=np.float32)
        for j in range(8):
            bias_d[:, j] = b_qkv[cols[j]]
            if j % 2 == 0:
                bias_d[:, j] *= scale
        bias_d[:, 8:] = np.tile(b_qkv[2 * C + s:2 * C + s + DL][None, :],
                                (P, 1))
        in_maps.append({
            "xb": np.ascontiguousarray(x[b].astype(bf16)),
            "wqk": wqk_d, "wv": wv_d, "wp": wp_d,
            "cs": cs, "bias": np.ascontiguousarray(bias_d), "tri": tri,
            "perm": pm,
        })
    return in_maps


_NC_CACHE = {}


def run(in_maps, **kwargs):
    if "nc" not in _NC_CACHE:
        _NC_CACHE["nc"] = build_nc()
    return run_bass_kernel_spmd(
        _NC_CACHE["nc"], in_maps, core_ids=list(range(NCORES)), **kwargs)


def kernel(x, W_qkv, b_qkv, W_proj, b_proj, **extra):
    in_maps = host_inputs(x, W_qkv, b_qkv, W_proj, b_proj)
    res = run(in_maps)
    b_proj = np.asarray(b_proj, dtype=np.float32)
    out = np.empty((B, T, C), dtype=np.float32)
    for b in range(B):
        out[b] = (res.results[2 * b]["out"].astype(np.float32)
                  + res.results[2 * b + 1]["out"].astype(np.float32) + b_proj)
    return out


# revision 58
# speedup vs baseline: 1.1845x; 1.0466x over previous
"""Trainium2 Bass kernel for multi-head causal attention with RoPE.

Problem: x[4,2048,1024] -> MHA(16 heads, head_dim 64, RoPE, causal) -> [4,2048,1024]

Sharding: 8 cores = 4 batches x 2 head-groups (8 heads each, Megatron-style).
Each core computes a partial [T, C] projection output for its batch; the host
sums the two head-group partials per batch and adds b_proj.

Per-core dataflow, chunked by 512-row t-blocks:
  A(tcn): x^T via DMA-engine xbar transposes (HBM -> SBUF, zero PE/DVE cost),
          Q^T/K^T bf16 GEMMs into a [2 heads x 64d] row layout with RoPE via
          a rot-half permutation matmul + elementwise muls; V in [t, h*64+e]
          bf16 with the qkv bias fused into the PSUM->SBUF copy
  B(qc=tcn): per head-pair (2g, 2g+1): scores S^T = K Q^T (bf16, K=64);
          causal masking of diagonal blocks by a -64 triangular matmul
          accumulated into the score PSUM pre-exp (no vector mask work);
          exp on ACT to bf16; PV flipped (P^T stationary, V moving) so all
          128 output partitions are useful y[q, e]; softmax denominator via
          a ones-column matmul; accumulation groups share a PSUM bank, so
          banks are pre-zeroed and all PV matmuls run with start=False
          (a start=True would mark the whole 2KB bank pending-zero and
          clobber sibling groups); normalize = one reciprocal + broadcast
          multiply per head pair; y -> y^T via one SBUF xbar-transpose DMA
          per chunk
  C(tcn): y^T @ W_proj (bf16), one batched out DMA per chunk

DMA instruction count is minimized (19 total): in this cost model each HWDGE
DMA serializes globally at ~3us (dge delay + transfer + sem propagation), so
per-chunk batching of the x-transpose, y-transpose and output store matters
more than transfer size. Weight loads are ordered by first use so the PE can
start ~6us in; chunk-0's slot-1 QK weights ride the b_phase(0) feed.
"""

import math
import sys

import numpy as np

if "/opt/trn_rl_repo" not in sys.path:
    sys.path.insert(0, "/opt/trn_rl_repo")

import concourse.bass as bass
import concourse.tile as tile
from concourse import bacc
from concourse import mybir
from concourse.bass_utils import run_bass_kernel_spmd
from concourse.masks import make_identity

B, T, C = 4, 2048, 1024
NH, D = 16, 64
HL = 8              # local heads per core
DL = HL * D         # 512
NCORES = 8
P = 128
TCH = 512           # t-chunk width
NTC = T // TCH
ROPE_BASE = 10000.0

F32 = mybir.dt.float32
BF16 = mybir.dt.bfloat16
Exp = mybir.ActivationFunctionType.Exp
Mul = mybir.AluOpType.mult
Add = mybir.AluOpType.add


def _emit(tc, xb, wqk, wv, wp, cs, bias, tri, perm, out, dbg=None):
    nc = tc.nc
    with tc.tile_pool(name="pers", bufs=1) as pers:
        wqk_sb = pers.tile([P, 8, 8, P], BF16)    # [p, j, cc, n]
        wv_sb = pers.tile([P, 8, DL], BF16)       # [p, cc, h*64+e]
        wp_sb = pers.tile([P, 4, C], BF16)        # [e2, g, n]
        cs_sb = pers.tile([P, 2, T], BF16)        # cos/sin, row r -> freq r%32
        bias_sb = pers.tile([P, 8 + DL], F32)
        tri_sb = pers.tile([P, P], BF16)          # tri[c,k] = -64*[c<k]
        perm_sb = pers.tile([P, P], BF16)         # rot-half permutation
        ident = pers.tile([P, P], BF16)
        make_identity(nc, ident)
        oneb = pers.tile([P, 1], BF16)
        nc.vector.memset(oneb[:], 1.0)
        # Q^T/K^T bf16: [row = h2*64 + d, j = 2g + kind (Q/K of pair g), t]
        qkb = pers.tile([P, 8, T], BF16)
        # V bf16: [t%128, t-tile, head, e | ones]; col 64 feeds the
        # softmax denominator through the same PV matmul
        vsb = pers.tile([P, 16, HL, D + 1], BF16)
        nc.vector.memset(vsb.rearrange("p a b c -> p (a b) c")[:, :, D:], 1.0)
        bias_v = bias_sb[:, 8:].rearrange("p (h e) -> p h e", e=D)

        # first-use-ordered weight loads; xT chunk-0 transpose interleaves
        nc.scalar.dma_start(wqk_sb[:, 0:2],
                            wqk[0:2].rearrange("j p cc n -> p j cc n"))

        with tc.tile_pool(name="xT", bufs=3) as pxT, \
             tc.tile_pool(name="t1", bufs=3) as pt1, \
             tc.tile_pool(name="tmp", bufs=4) as ptmp, \
             tc.tile_pool(name="pt", bufs=6) as ppt, \
             tc.tile_pool(name="y", bufs=2) as py, \
             tc.tile_pool(name="rcp", bufs=2) as prcp, \
             tc.tile_pool(name="yT", bufs=2) as pyT, \
             tc.tile_pool(name="ost", bufs=2) as post, \
             tc.tile_pool(name="psS", bufs=2, space="PSUM") as psS, \
             tc.tile_pool(name="psQ", bufs=2, space="PSUM") as psQ, \
             tc.tile_pool(name="psO", bufs=2, space="PSUM") as psO_p:

            def a_units(tcn):
                """Chunk tcn's QKV phase: xbar-transpose DMA, QK GEMM+RoPE,
                V GEMM. Units sized ~0.9us for fine interleaving."""
                ts0 = tcn * TCH
                xT = pxT.tile([P, 8, TCH], BF16, tag="xT")
                units = []

                def xt_unit(xT=xT, ts0=ts0):
                    nc.sync.dma_start_transpose(
                        xT[:], xb[ts0: ts0 + TCH, :])
                xt_list = [xt_unit]

                qk_st = {}

                def qk_half_a(j, xT=xT):
                    psq = psQ.tile([P, TCH], F32, tag="q")
                    qk_st[j] = psq
                    for cc in range(4):
                        nc.tensor.matmul(
                            psq[:],
                            wqk_sb[:, j, cc, :],
                            xT[:, cc, :],
                            start=(cc == 0), stop=False)

                def qk_unit(j, xT=xT, ts0=ts0):
                    psq = qk_st.pop(j)
                    for cc in range(4, 8):
                        nc.tensor.matmul(
                            psq[:],
                            wqk_sb[:, j, cc, :],
                            xT[:, cc, :],
                            start=False, stop=(cc == 7))
                    t1 = pt1.tile([P, TCH], BF16, tag="t1")
                    nc.vector.tensor_scalar_add(t1[:], psq[:],
                                                bias_sb[:, j:j + 1])
                    # psq is dead after the bias copy; reuse its bank for
                    # the rot-half permutation product (keeps psQ at one
                    # allocation per unit so the 2-buf ring never wraps
                    # onto a live tile)
                    nc.tensor.matmul(psq[:], perm_sb[:], t1[:],
                                     start=True, stop=True,
                                     skip_group_check=True)
                    dst = qkb[:, j, ts0:ts0 + TCH]
                    nc.vector.tensor_tensor(dst, t1[:],
                                            cs_sb[:, 0, ts0:ts0 + TCH], Mul)
                    swp = ptmp.tile([P, TCH], BF16, tag="tmp")
                    nc.vector.tensor_tensor(swp[:], psq[:],
                                            cs_sb[:, 1, ts0:ts0 + TCH], Mul)
                    nc.vector.tensor_tensor(dst, dst, swp[:], Add)
                for j in range(4):
                    units.append(lambda j=j: qk_half_a(j))
                    units.append(lambda j=j: qk_unit(j))

                def v_half_a(i, xT=xT):
                    psv = psQ.tile([P, DL], F32, tag="q")
                    qk_st[8 + i] = psv
                    for cc in range(4):
                        nc.tensor.matmul(
                            psv[:],
                            xT[:, cc, i * P:(i + 1) * P],
                            wv_sb[:, cc, :],
                            start=(cc == 0), stop=False)

                def v_unit(i, xT=xT, tcn=tcn):
                    ti = tcn * (TCH // P) + i
                    psv = qk_st.pop(8 + i)
                    for cc in range(4, 8):
                        nc.tensor.matmul(
                            psv[:],
                            xT[:, cc, i * P:(i + 1) * P],
                            wv_sb[:, cc, :],
                            start=False, stop=(cc == 7))
                    psvh = psv.rearrange("p (h e) -> p h e", e=D)
                    nc.vector.tensor_tensor(vsb[:, ti, :, 0:D], psvh,
                                            bias_v, Add)
                for i in range(TCH // P):
                    units.append(lambda i=i: v_half_a(i))
                    units.append(lambda i=i: v_unit(i))
                for j in range(4, 8):
                    units.append(lambda j=j: qk_half_a(j))
                    units.append(lambda j=j: qk_unit(j))
                return xt_list, units

            def c_units(tcn, yTt):
                """Projection for chunk tcn; needs yTt complete."""
                ts0 = tcn * TCH
                units = []

                ost = post.tile([P, 4, C], BF16, tag="ost",
                                name=f"ost_{tcn}")

                def c_unit(i, n, yTt=yTt, ost=ost):
                    psp = psQ.tile([P, 512], F32, tag="q")
                    for g in range(4):
                        nc.tensor.matmul(
                            psp[:],
                            yTt[:, g, i * P:(i + 1) * P],
                            wp_sb[:, g, n * 512:(n + 1) * 512],
                            start=(g == 0), stop=(g == 3))
                    nc.vector.tensor_copy(
                        ost[:, i, n * 512:(n + 1) * 512], psp[:])

                def c_flush(ost=ost, ts0=ts0):
                    nc.sync.dma_start(
                        out[ts0: ts0 + TCH, :].rearrange(
                            "(i p) c -> p i c", p=P), ost[:])
                for i in range(TCH // P):
                    for n in range(2):
                        units.append(lambda i=i, n=n: c_unit(i, n))
                units.append(c_flush)
                return units

            def b_phase(qc, feed, pre):
                """Attention for q-chunk qc; drains `feed` units into PE
                slack while ACT exps. `pre` = next chunk's x-transpose
                (urgent, wait-free) then the previous chunk's y->yT
                transpose."""
                for f in pre:
                    f()
                nblk = 4 * qc + 4
                yTt = pyT.tile([P, 4, TCH], BF16, tag="yT")
                ych = py.tile([P, 4, 4, P], BF16, tag="y",
                              name=f"ych_{qc}")
                drain = {"done": 0, "seen": 0, "n0": max(1, len(feed))}
                total_kc = 4 * nblk

                def drain_tick():
                    # spread the feed evenly over the phase's kc steps
                    drain["seen"] += 1
                    want = drain["seen"] * drain["n0"] // total_kc
                    while feed and drain["done"] < want:
                        feed.pop(0)()
                        drain["done"] += 1
                for g in range(4):
                    psO0 = psO_p.tile([P, 4, D + 1], F32, tag="o",
                                      name=f"psO0_{qc}_{g}")
                    psO1 = psO_p.tile([P, 4, D + 1], F32, tag="o",
                                      name=f"psO1_{qc}_{g}")
                    psO = (psO0, psO1)
                    # pre-zero: 4 accumulation groups share each bank; a
                    # start=True would mark the whole 2KB bank pending-zero
                    # and clobber sibling groups, so accumulate-only.
                    nc.vector.memset(psO0[:], 0.0)
                    nc.vector.memset(psO1[:], 0.0)
                    pv_q = []          # (kc, pt) with 3-block lag

                    def pv_blk(kc, pt, qc=qc, g=g, psO=psO):
                        for hh in range(2):
                            head = 2 * g + hh
                            for qi in range(4):
                                last_kc = 4 * qc + qi
                                if kc > last_kc:
                                    continue
                                stop = (kc == last_kc)
                                lhs = pt[:, hh * 512 + qi * P:
                                         hh * 512 + (qi + 1) * P]
                                nc.tensor.matmul(
                                    psO[hh][:, qi, :], lhs,
                                    vsb[:, kc, head, :],
                                    start=False, stop=stop,
                                    skip_group_check=True)

                    for kc in range(nblk):
                        if qc == 0 and feed and kc < 2:
                            feed.pop(0)()     # chunk-0 carry must emit early
                        elif kc >= 2:
                            drain_tick()
                        m = kc - 4 * qc
                        pt = ppt.tile([P, 1024], BF16, tag="pt",
                                      name=f"pt_{qc}_{g}_{kc}")
                        pss = psS.tile([P, 1024], F32, tag="s")
                        for hh in range(2):
                            q0 = m * P if m > 0 else 0
                            nc.tensor.matmul(
                                pss[:, hh * 512 + q0:(hh + 1) * 512],
                                qkb[64 * hh:64 * hh + 64, 2 * g + 1,
                                    kc * P:(kc + 1) * P],
                                qkb[64 * hh:64 * hh + 64, 2 * g,
                                    qc * TCH + q0:(qc + 1) * TCH],
                                start=True, stop=(m < 0),
                                skip_group_check=True,
                                tile_position=(64 * hh, 0))
                            if m >= 0:
                                nc.tensor.matmul(
                                    pss[:, hh * 512 + q0:
                                        hh * 512 + q0 + P],
                                    tri_sb[:], ident[:],
                                    start=False, stop=True,
                                    skip_group_check=True)
                        with nc.allow_low_precision(reason="bf16 softmax"):
                            if m < 0:
                                nc.scalar.activation(pt[:], pss[:], Exp)
                            else:
                                q0 = m * P
                                for hh in range(2):
                                    nc.scalar.activation(
                                        pt[:, hh * 512 + q0:(hh + 1) * 512],
                                        pss[:, hh * 512 + q0:(hh + 1) * 512],
                                        Exp)
                        if len(pv_q) >= 4:
                            pv_blk(*pv_q.pop(0))
                        pv_q.append((kc, pt))
                    while pv_q:
                        pv_blk(*pv_q.pop(0))

                    rcp = prcp.tile([P, 2, 4], F32, tag="rcp")
                    yv = ych[:, g].rearrange("p a (h e) -> p h a e", h=2)
                    for hh in range(2):
                        nc.vector.reciprocal(rcp[:, hh],
                                             psO[hh][:, :, D])
                        nc.vector.tensor_tensor(
                            yv[:, hh], psO[hh][:, :, 0:D],
                            rcp[:, hh].rearrange(
                                "p (a o) -> p a o", o=1).to_broadcast(
                                    (P, 4, D)),
                            Mul)
                    if feed and g < 3:
                        feed.pop(0)()
                while feed:
                    feed.pop(0)()

                def ytr(ych=ych, yTt=yTt):
                    nc.sync.dma_start_transpose(
                        yTt.rearrange("p g (a c) -> p (g a) c", c=P),
                        ych[:])
                return yTt, [ytr]

            xt0, a0 = a_units(0)
            nc.scalar.dma_start(bias_sb[:], bias)
            nc.scalar.dma_start(tri_sb[:], tri)
            nc.scalar.dma_start(perm_sb[:], perm)
            for u in xt0:              # x^T chunk-0 transpose (SP queue)
                u()
            nc.scalar.dma_start(cs_sb[:], cs)
            nc.scalar.dma_start(wqk_sb[:, 2:4],
                                wqk[2:4].rearrange("j p cc n -> p j cc n"))
            for u in a0[:8]:           # qk of head-pairs 0,1 (j=0..3)
                u()
            nc.sync.dma_start(wv_sb[:], wv)
            nc.scalar.dma_start(wqk_sb[:, 4:8],
                                wqk[4:8].rearrange("j p cc n -> p j cc n"))
            nc.sync.dma_start(wp_sb[:], wp)
            for u in a0[8:16]:         # v units (needed by b_phase(0) PV)
                u()
            # chunk-0 QK of head-pairs 2,3 ride as b_phase(0) feed: their
            # weights (wqk[4:8]) land late in the DMA chain and must not
            # block the first head-pairs' scores in PE program order; they
            # drain before g=2 needs them
            yT_prev, ytr_prev = None, []
            carry = a0[16:]
            for tcn in range(NTC):
                feed = list(carry)
                carry = []
                pre = []
                if yT_prev is not None:
                    feed.extend(c_units(tcn - 1, yT_prev))
                if tcn + 1 < NTC:
                    xt_n, a_n = a_units(tcn + 1)
                    pre.extend(xt_n)
                    feed.extend(a_n)
                pre.extend(ytr_prev)
                yT_prev, ytr_prev = b_phase(tcn, feed, pre)
            for f in ytr_prev:
                f()
            for u in c_units(NTC - 1, yT_prev):
                u()
            if dbg is not None:
                nc.sync.dma_start(dbg["qkb"], qkb[:])
                nc.sync.dma_start(dbg["vsb"], vsb[:])
                nc.sync.dma_start(dbg["yT3"], yT_prev[:])


def build_nc(debug=False):
    nc = bacc.Bacc("TRN2", target_bir_lowering=False, debug=False)
    xb = nc.dram_tensor("xb", [T, C], BF16, kind="ExternalInput").ap()
    wqk = nc.dram_tensor("wqk", [8, P, 8, P], BF16, kind="ExternalInput").ap()
    wv = nc.dram_tensor("wv", [P, 8, DL], BF16, kind="ExternalInput").ap()
    wp = nc.dram_tensor("wp", [P, 4, C], BF16, kind="ExternalInput").ap()
    cs = nc.dram_tensor("cs", [P, 2, T], BF16, kind="ExternalInput").ap()
    bias = nc.dram_tensor("bias", [P, 8 + DL], F32, kind="ExternalInput").ap()
    tri = nc.dram_tensor("tri", [P, P], BF16, kind="ExternalInput").ap()
    perm = nc.dram_tensor("perm", [P, P], BF16, kind="ExternalInput").ap()
    out = nc.dram_tensor("out", [T, C], BF16, kind="ExternalOutput").ap()
    dbg = None
    if debug:
        dbg = {
            "qkb": nc.dram_tensor("d_qkb", [P, 8, T], BF16,
                                  kind="ExternalOutput").ap(),
            "vsb": nc.dram_tensor("d_vsb", [P, 16, HL, D], BF16,
                                  kind="ExternalOutput").ap(),
            "yT3": nc.dram_tensor("d_yT3", [P, 4, TCH], BF16,
                                  kind="ExternalOutput").ap(),
        }
    with tile.TileContext(nc) as tc:
        _emit(tc, xb, wqk, wv, wp, cs, bias, tri, perm, out, dbg=dbg)
    nc.compile()
    return nc


def rope_tables():
    inv_freq = 1.0 / (ROPE_BASE ** (np.arange(0, D, 2, dtype=np.float64) / D))
    t = np.arange(T, dtype=np.float64)
    freqs = np.outer(t, inv_freq)                      # [T, 32]
    cosT = np.cos(freqs).T.astype(np.float32)          # [32, T]
    sinT = np.sin(freqs).T.astype(np.float32)
    cos4 = np.tile(cosT, (4, 1))                       # [128, T]
    sin4 = np.tile(sinT, (4, 1))
    return np.ascontiguousarray(np.stack([cos4, sin4], axis=1))  # [128,2,T]


def perm_matrix():
    pm = np.zeros((P, P), dtype=np.float32)
    for base in (0, 64):
        for d in range(32):
            pm[base + d + 32, base + d] = -1.0       # rot_half: -x2 into top
            pm[base + d, base + d + 32] = 1.0        # +x1 into bottom
    return pm


def host_inputs(x, W_qkv, b_qkv, W_proj, b_proj):
    import ml_dtypes
    bf16 = ml_dtypes.bfloat16
    x = np.asarray(x, dtype=np.float32)
    W_qkv = np.asarray(W_qkv, dtype=np.float32)
    b_qkv = np.asarray(b_qkv, dtype=np.float32)
    W_proj = np.asarray(W_proj, dtype=np.float32)
    scale = 1.0 / math.sqrt(D)
    cs = rope_tables().astype(bf16)
    tri = np.zeros((P, P), dtype=np.float32)
    for c_ in range(P):
        tri[c_, c_ + 1:] = -64.0
    tri = np.ascontiguousarray(tri.astype(bf16))
    pm = np.ascontiguousarray(perm_matrix().astype(bf16))

    in_maps = []
    for core in range(NCORES):
        b = core // 2
        hg = core % 2
        s = hg * DL
        # wqk: [j, p, cc, n]; j = 2g + kind; psq row n = h2*64 + d;
        # W col = kind*C + s + (2g + h2)*64 + d ; input channel = cc*128+p
        cols = np.empty((8, P), dtype=np.int64)
        for j in range(8):
            g, kind = j // 2, j % 2
            for n in range(P):
                h2, d = n // 64, n % 64
                cols[j, n] = kind * C + s + (2 * g + h2) * 64 + d
        wqk_d = np.empty((8, P, 8, P), dtype=np.float32)
        for j in range(8):
            wcols = W_qkv[:, cols[j]]                    # [1024, 128]
            if j % 2 == 0:                               # Q: fold 1/sqrt(D)
                wcols = wcols * scale
            wqk_d[j] = wcols.reshape(8, P, P).transpose(1, 0, 2)
        wqk_d = np.ascontiguousarray(wqk_d.astype(bf16))
        wv_f = W_qkv[:, 2 * C + s:2 * C + s + DL]        # [1024, 512]
        wv_d = np.ascontiguousarray(
            wv_f.reshape(8, P, DL).transpose(1, 0, 2).astype(bf16))
        # wp rows: e2-row p of g-tile = head 2g + p//64, e = p%64
        wp_d = np.empty((P, 4, C), dtype=np.float32)
        for g in range(4):
            for p_ in range(P):
                head = 2 * g + (p_ // 64)
                wp_d[p_, g] = W_proj[s + head * 64 + (p_ % 64), :]
        wp_d = np.ascontiguousarray(wp_d.astype(bf16))
        bias_d = np.zeros((P, 8 + DL), dtype=np.float32)
        for j in range(8):
            bias_d[:, j] = b_qkv[cols[j]]
            if j % 2 == 0:
                bias_d[:, j] *= scale
        bias_d[:, 8:] = np.tile(b_qkv[2 * C + s:2 * C + s + DL][None, :],
                                (P, 1))
        in_maps.append({
            "xb": np.ascontiguousarray(x[b].astype(bf16)),
            "wqk": wqk_d, "wv": wv_d, "wp": wp_d,
            "cs": cs, "bias": np.ascontiguousarray(bias_d), "tri": tri,
            "perm": pm,
        })
    return in_maps


_NC_CACHE = {}


def run(in_maps, **kwargs):
    if "nc" not in _NC_CACHE:
        _NC_CACHE["nc"] = build_nc()
    return run_bass_kernel_spmd(
        _NC_CACHE["nc"], in_maps, core_ids=list(range(NCORES)), **kwargs)


def kernel(x, W_qkv, b_qkv, W_proj, b_proj, **extra):
    in_maps = host_inputs(x, W_qkv, b_qkv, W_proj, b_proj)
    res = run(in_maps)
    b_proj = np.asarray(b_proj, dtype=np.float32)
    out = np.empty((B, T, C), dtype=np.float32)
    for b in range(B):
        out[b] = (res.results[2 * b]["out"].astype(np.float32)
                  + res.results[2 * b + 1]["out"].astype(np.float32) + b_proj)
    return out


# revision 59
# speedup vs baseline: 1.2043x; 1.0168x over previous
"""Trainium2 Bass kernel for multi-head causal attention with RoPE.

Problem: x[4,2048,1024] -> MHA(16 heads, head_dim 64, RoPE, causal) -> [4,2048,1024]

Sharding: 8 cores = 4 batches x 2 head-groups (8 heads each, Megatron-style).
Each core computes a partial [T, C] projection output for its batch; the host
sums the two head-group partials per batch and adds b_proj.

Per-core dataflow, chunked by 512-row t-blocks:
  A(tcn): x^T via DMA-engine xbar transposes (HBM -> SBUF, zero PE/DVE cost),
          Q^T/K^T bf16 GEMMs into a [2 heads x 64d] row layout with RoPE via
          a rot-half permutation matmul + elementwise muls; V in [t, h*64+e]
          bf16 with the qkv bias fused into the PSUM->SBUF copy
  B(qc=tcn): per head-pair (2g, 2g+1): scores S^T = K Q^T (bf16, K=64);
          causal masking of diagonal blocks by a -64 triangular matmul
          accumulated into the score PSUM pre-exp (no vector mask work);
          exp on ACT to bf16; PV flipped (P^T stationary, V moving) so all
          128 output partitions are useful y[q, e]; softmax denominator via
          a ones-column matmul; accumulation groups share a PSUM bank, so
          banks are pre-zeroed and all PV matmuls run with start=False
          (a start=True would mark the whole 2KB bank pending-zero and
          clobber sibling groups); normalize = one reciprocal + broadcast
          multiply per head pair; y -> y^T via one SBUF xbar-transpose DMA
          per chunk
  C(tcn): y^T @ W_proj (bf16), one batched out DMA per chunk

DMA instruction count is minimized (19 total): in this cost model each HWDGE
DMA serializes globally at ~3us (dge delay + transfer + sem propagation), so
per-chunk batching of the x-transpose, y-transpose and output store matters
more than transfer size. Weight loads are ordered by first use so the PE can
start ~6us in; chunk-0's slot-1 QK weights ride the b_phase(0) feed.
"""

import math
import sys

import numpy as np

if "/opt/trn_rl_repo" not in sys.path:
    sys.path.insert(0, "/opt/trn_rl_repo")

import concourse.bass as bass
import concourse.tile as tile
from concourse import bacc
from concourse import mybir
from concourse.bass_utils import run_bass_kernel_spmd
from concourse.masks import make_identity

B, T, C = 4, 2048, 1024
NH, D = 16, 64
HL = 8              # local heads per core
DL = HL * D         # 512
NCORES = 8
P = 128
TCH = 512           # t-chunk width
NTC = T // TCH
ROPE_BASE = 10000.0

F32 = mybir.dt.float32
BF16 = mybir.dt.bfloat16
Exp = mybir.ActivationFunctionType.Exp
Mul = mybir.AluOpType.mult
Add = mybir.AluOpType.add


def _emit(tc, xb, wqk, wv, wp, cs, bias, tri, perm, out, dbg=None):
    nc = tc.nc
    with tc.tile_pool(name="pers", bufs=1) as pers:
        wqk_sb = pers.tile([P, 8, 8, P], BF16)    # [p, j, cc, n]
        wv_sb = pers.tile([P, 8, DL], BF16)       # [p, cc, h*64+e]
        wp_sb = pers.tile([P, 4, C], BF16)        # [e2, g, n]
        cs_sb = pers.tile([P, 2, T], BF16)        # cos/sin, row r -> freq r%32
        bias_sb = pers.tile([P, 8 + DL], F32)
        tri_sb = pers.tile([P, P], BF16)          # tri[c,k] = -64*[c<k]
        perm_sb = pers.tile([P, P], BF16)         # rot-half permutation
        ident = pers.tile([P, P], BF16)
        make_identity(nc, ident)
        oneb = pers.tile([P, 1], BF16)
        nc.vector.memset(oneb[:], 1.0)
        # Q^T/K^T bf16: [row = h2*64 + d, j = 2g + kind (Q/K of pair g), t]
        qkb = pers.tile([P, 8, T], BF16)
        # V bf16: [t%128, t-tile, head, e | ones]; col 64 feeds the
        # softmax denominator through the same PV matmul
        vsb = pers.tile([P, 16, HL, D + 1], BF16)
        nc.vector.memset(vsb.rearrange("p a b c -> p (a b) c")[:, :, D:], 1.0)
        bias_v = bias_sb[:, 8:].rearrange("p (h e) -> p h e", e=D)

        # first-use-ordered weight loads; xT chunk-0 transpose interleaves
        nc.scalar.dma_start(wqk_sb[:, 0:2],
                            wqk[0:2].rearrange("j p cc n -> p j cc n"))

        with tc.tile_pool(name="xT", bufs=3) as pxT, \
             tc.tile_pool(name="t1", bufs=3) as pt1, \
             tc.tile_pool(name="tmp", bufs=4) as ptmp, \
             tc.tile_pool(name="pt", bufs=6) as ppt, \
             tc.tile_pool(name="y", bufs=2) as py, \
             tc.tile_pool(name="rcp", bufs=2) as prcp, \
             tc.tile_pool(name="yT", bufs=2) as pyT, \
             tc.tile_pool(name="ost", bufs=2) as post, \
             tc.tile_pool(name="psS", bufs=2, space="PSUM") as psS, \
             tc.tile_pool(name="psQ", bufs=2, space="PSUM") as psQ, \
             tc.tile_pool(name="psO", bufs=2, space="PSUM") as psO_p:

            def a_units(tcn):
                """Chunk tcn's QKV phase: xbar-transpose DMA, QK GEMM+RoPE,
                V GEMM. Units sized ~0.9us for fine interleaving."""
                ts0 = tcn * TCH
                xT = pxT.tile([P, 8, TCH], BF16, tag="xT")
                units = []

                def xt_unit(xT=xT, ts0=ts0):
                    nc.sync.dma_start_transpose(
                        xT[:], xb[ts0: ts0 + TCH, :])
                xt_list = [xt_unit]

                qk_st = {}

                def qk_half_a(j, xT=xT):
                    psq = psQ.tile([P, TCH], F32, tag="q")
                    qk_st[j] = psq
                    for cc in range(4):
                        nc.tensor.matmul(
                            psq[:],
                            wqk_sb[:, j, cc, :],
                            xT[:, cc, :],
                            start=(cc == 0), stop=False)

                def qk_unit(j, xT=xT, ts0=ts0):
                    psq = qk_st.pop(j)
                    for cc in range(4, 8):
                        nc.tensor.matmul(
                            psq[:],
                            wqk_sb[:, j, cc, :],
                            xT[:, cc, :],
                            start=False, stop=(cc == 7))
                    t1 = pt1.tile([P, TCH], BF16, tag="t1")
                    nc.vector.tensor_scalar_add(t1[:], psq[:],
                                                bias_sb[:, j:j + 1])
                    # psq is dead after the bias copy; reuse its bank for
                    # the rot-half permutation product (keeps psQ at one
                    # allocation per unit so the 2-buf ring never wraps
                    # onto a live tile)
                    nc.tensor.matmul(psq[:], perm_sb[:], t1[:],
                                     start=True, stop=True,
                                     skip_group_check=True)
                    dst = qkb[:, j, ts0:ts0 + TCH]
                    nc.vector.tensor_tensor(dst, t1[:],
                                            cs_sb[:, 0, ts0:ts0 + TCH], Mul)
                    swp = ptmp.tile([P, TCH], BF16, tag="tmp")
                    nc.vector.tensor_tensor(swp[:], psq[:],
                                            cs_sb[:, 1, ts0:ts0 + TCH], Mul)
                    nc.vector.tensor_tensor(dst, dst, swp[:], Add)
                for j in range(4):
                    units.append(lambda j=j: qk_half_a(j))
                    units.append(lambda j=j: qk_unit(j))

                def v_half_a(i, xT=xT):
                    psv = psQ.tile([P, DL], F32, tag="q")
                    qk_st[8 + i] = psv
                    for cc in range(4):
                        nc.tensor.matmul(
                            psv[:],
                            xT[:, cc, i * P:(i + 1) * P],
                            wv_sb[:, cc, :],
                            start=(cc == 0), stop=False)

                def v_unit(i, xT=xT, tcn=tcn):
                    ti = tcn * (TCH // P) + i
                    psv = qk_st.pop(8 + i)
                    for cc in range(4, 8):
                        nc.tensor.matmul(
                            psv[:],
                            xT[:, cc, i * P:(i + 1) * P],
                            wv_sb[:, cc, :],
                            start=False, stop=(cc == 7))
                    psvh = psv.rearrange("p (h e) -> p h e", e=D)
                    nc.vector.tensor_tensor(vsb[:, ti, :, 0:D], psvh,
                                            bias_v, Add)
                for i in range(TCH // P):
                    units.append(lambda i=i: v_half_a(i))
                    units.append(lambda i=i: v_unit(i))
                for j in range(4, 8):
                    units.append(lambda j=j: qk_half_a(j))
                    units.append(lambda j=j: qk_unit(j))
                return xt_list, units

            def c_units(tcn, yTt):
                """Projection for chunk tcn; needs yTt complete."""
                ts0 = tcn * TCH
                units = []

                ost = post.tile([P, 4, C], BF16, tag="ost",
                                name=f"ost_{tcn}")

                def c_unit(i, n, yTt=yTt, ost=ost):
                    psp = psQ.tile([P, 512], F32, tag="q")
                    for g in range(4):
                        nc.tensor.matmul(
                            psp[:],
                            yTt[:, g, i * P:(i + 1) * P],
                            wp_sb[:, g, n * 512:(n + 1) * 512],
                            start=(g == 0), stop=(g == 3))
                    nc.vector.tensor_copy(
                        ost[:, i, n * 512:(n + 1) * 512], psp[:])

                def c_flush_half(h, ost=ost, ts0=ts0):
                    nc.sync.dma_start(
                        out[ts0 + h * 256: ts0 + (h + 1) * 256, :].rearrange(
                            "(i p) c -> p i c", p=P), ost[:, 2 * h:2 * h + 2])

                def c_flush(ost=ost, ts0=ts0):
                    nc.sync.dma_start(
                        out[ts0: ts0 + TCH, :].rearrange(
                            "(i p) c -> p i c", p=P), ost[:])
                last = (tcn == NTC - 1)
                for i in range(TCH // P):
                    for n in range(2):
                        units.append(lambda i=i, n=n: c_unit(i, n))
                        if last and i == 1 and n == 1:
                            units.append(lambda: c_flush_half(0))
                if last:
                    units.append(lambda: c_flush_half(1))
                else:
                    units.append(c_flush)
                return units

            def b_phase(qc, feed, pre):
                """Attention for q-chunk qc; drains `feed` units into PE
                slack while ACT exps. `pre` = next chunk's x-transpose
                (urgent, wait-free) then the previous chunk's y->yT
                transpose."""
                for f in pre:
                    f()
                nblk = 4 * qc + 4
                yTt = pyT.tile([P, 4, TCH], BF16, tag="yT")
                ych = py.tile([P, 4, 4, P], BF16, tag="y",
                              name=f"ych_{qc}")
                drain = {"done": 0, "seen": 0, "n0": max(1, len(feed))}
                total_kc = 4 * nblk

                def drain_tick():
                    # spread the feed evenly over the phase's kc steps
                    drain["seen"] += 1
                    want = drain["seen"] * drain["n0"] // total_kc
                    while feed and drain["done"] < want:
                        feed.pop(0)()
                        drain["done"] += 1
                for g in range(4):
                    psO0 = psO_p.tile([P, 4, D + 1], F32, tag="o",
                                      name=f"psO0_{qc}_{g}")
                    psO1 = psO_p.tile([P, 4, D + 1], F32, tag="o",
                                      name=f"psO1_{qc}_{g}")
                    psO = (psO0, psO1)
                    # pre-zero: 4 accumulation groups share each bank; a
                    # start=True would mark the whole 2KB bank pending-zero
                    # and clobber sibling groups, so accumulate-only.
                    nc.vector.memset(psO0[:], 0.0)
                    nc.vector.memset(psO1[:], 0.0)
                    pv_q = []          # (kc, pt) with 3-block lag

                    def pv_blk(kc, pt, qc=qc, g=g, psO=psO):
                        for hh in range(2):
                            head = 2 * g + hh
                            for qi in range(4):
                                last_kc = 4 * qc + qi
                                if kc > last_kc:
                                    continue
                                stop = (kc == last_kc)
                                lhs = pt[:, hh * 512 + qi * P:
                                         hh * 512 + (qi + 1) * P]
                                nc.tensor.matmul(
                                    psO[hh][:, qi, :], lhs,
                                    vsb[:, kc, head, :],
                                    start=False, stop=stop,
                                    skip_group_check=True)

                    for kc in range(nblk):
                        if qc == 0 and feed and kc < 2:
                            feed.pop(0)()     # chunk-0 carry must emit early
                        elif kc >= 2:
                            drain_tick()
                        m = kc - 4 * qc
                        pt = ppt.tile([P, 1024], BF16, tag="pt",
                                      name=f"pt_{qc}_{g}_{kc}")
                        pss = psS.tile([P, 1024], F32, tag="s")
                        for hh in range(2):
                            q0 = m * P if m > 0 else 0
                            nc.tensor.matmul(
                                pss[:, hh * 512 + q0:(hh + 1) * 512],
                                qkb[64 * hh:64 * hh + 64, 2 * g + 1,
                                    kc * P:(kc + 1) * P],
                                qkb[64 * hh:64 * hh + 64, 2 * g,
                                    qc * TCH + q0:(qc + 1) * TCH],
                                start=True, stop=(m < 0),
                                skip_group_check=True,
                                tile_position=(64 * hh, 0))
                            if m >= 0:
                                nc.tensor.matmul(
                                    pss[:, hh * 512 + q0:
                                        hh * 512 + q0 + P],
                                    tri_sb[:], ident[:],
                                    start=False, stop=True,
                                    skip_group_check=True)
                        with nc.allow_low_precision(reason="bf16 softmax"):
                            if m < 0:
                                nc.scalar.activation(pt[:], pss[:], Exp)
                            else:
                                q0 = m * P
                                for hh in range(2):
                                    nc.scalar.activation(
                                        pt[:, hh * 512 + q0:(hh + 1) * 512],
                                        pss[:, hh * 512 + q0:(hh + 1) * 512],
                                        Exp)
                        if len(pv_q) >= 4:
                            pv_blk(*pv_q.pop(0))
                        pv_q.append((kc, pt))
                    while pv_q:
                        pv_blk(*pv_q.pop(0))

                    rcp = prcp.tile([P, 2, 4], F32, tag="rcp")
                    yv = ych[:, g].rearrange("p a (h e) -> p h a e", h=2)
                    for hh in range(2):
                        nc.vector.reciprocal(rcp[:, hh],
                                             psO[hh][:, :, D])
                        nc.vector.tensor_tensor(
                            yv[:, hh], psO[hh][:, :, 0:D],
                            rcp[:, hh].rearrange(
                                "p (a o) -> p a o", o=1).to_broadcast(
                                    (P, 4, D)),
                            Mul)
                    if feed and g < 3:
                        feed.pop(0)()
                    if qc == NTC - 1 and g == 1:
                        # last chunk: transpose the first half early so the
                        # final projection's g0/g1 matmuls start immediately
                        nc.sync.dma_start_transpose(
                            yTt[:, 0:2].rearrange(
                                "p g (a c) -> p (g a) c", c=P),
                            ych[:, 0:2])
                while feed:
                    feed.pop(0)()

                if qc == NTC - 1:
                    def ytr(ych=ych, yTt=yTt):
                        nc.sync.dma_start_transpose(
                            yTt[:, 2:4].rearrange(
                                "p g (a c) -> p (g a) c", c=P),
                            ych[:, 2:4])
                else:
                    def ytr(ych=ych, yTt=yTt):
                        nc.sync.dma_start_transpose(
                            yTt.rearrange("p g (a c) -> p (g a) c", c=P),
                            ych[:])
                return yTt, [ytr]

            xt0, a0 = a_units(0)
            nc.scalar.dma_start(bias_sb[:], bias)
            nc.scalar.dma_start(tri_sb[:], tri)
            nc.scalar.dma_start(perm_sb[:], perm)
            for u in xt0:              # x^T chunk-0 transpose (SP queue)
                u()
            nc.scalar.dma_start(cs_sb[:], cs)
            nc.scalar.dma_start(wqk_sb[:, 2:4],
                                wqk[2:4].rearrange("j p cc n -> p j cc n"))
            for u in a0[:8]:           # qk of head-pairs 0,1 (j=0..3)
                u()
            nc.sync.dma_start(wv_sb[:], wv)
            nc.scalar.dma_start(wqk_sb[:, 4:8],
                                wqk[4:8].rearrange("j p cc n -> p j cc n"))
            nc.sync.dma_start(wp_sb[:], wp)
            for u in a0[8:16]:         # v units (needed by b_phase(0) PV)
                u()
            # chunk-0 QK of head-pairs 2,3 ride as b_phase(0) feed: their
            # weights (wqk[4:8]) land late in the DMA chain and must not
            # block the first head-pairs' scores in PE program order; they
            # drain before g=2 needs them
            yT_prev, ytr_prev = None, []
            carry = a0[16:]
            for tcn in range(NTC):
                feed = list(carry)
                carry = []
                pre = []
                if yT_prev is not None:
                    feed.extend(c_units(tcn - 1, yT_prev))
                if tcn + 1 < NTC:
                    xt_n, a_n = a_units(tcn + 1)
                    pre.extend(xt_n)
                    feed.extend(a_n)
                pre.extend(ytr_prev)
                yT_prev, ytr_prev = b_phase(tcn, feed, pre)
            for f in ytr_prev:
                f()
            for u in c_units(NTC - 1, yT_prev):
                u()
            if dbg is not None:
                nc.sync.dma_start(dbg["qkb"], qkb[:])
                nc.sync.dma_start(dbg["vsb"], vsb[:])
                nc.sync.dma_start(dbg["yT3"], yT_prev[:])


def build_nc(debug=False):
    nc = bacc.Bacc("TRN2", target_bir_lowering=False, debug=False)
    xb = nc.dram_tensor("xb", [T, C], BF16, kind="ExternalInput").ap()
    wqk = nc.dram_tensor("wqk", [8, P, 8, P], BF16, kind="ExternalInput").ap()
    wv = nc.dram_tensor("wv", [P, 8, DL], BF16, kind="ExternalInput").ap()
    wp = nc.dram_tensor("wp", [P, 4, C], BF16, kind="ExternalInput").ap()
    cs = nc.dram_tensor("cs", [P, 2, T], BF16, kind="ExternalInput").ap()
    bias = nc.dram_tensor("bias", [P, 8 + DL], F32, kind="ExternalInput").ap()
    tri = nc.dram_tensor("tri", [P, P], BF16, kind="ExternalInput").ap()
    perm = nc.dram_tensor("perm", [P, P], BF16, kind="ExternalInput").ap()
    out = nc.dram_tensor("out", [T, C], BF16, kind="ExternalOutput").ap()
    dbg = None
    if debug:
        dbg = {
            "qkb": nc.dram_tensor("d_qkb", [P, 8, T], BF16,
                                  kind="ExternalOutput").ap(),
            "vsb": nc.dram_tensor("d_vsb", [P, 16, HL, D], BF16,
                                  kind="ExternalOutput").ap(),
            "yT3": nc.dram_tensor("d_yT3", [P, 4, TCH], BF16,
                                  kind="ExternalOutput").ap(),
        }
    with tile.TileContext(nc) as tc:
        _emit(tc, xb, wqk, wv, wp, cs, bias, tri, perm, out, dbg=dbg)
    nc.compile()
    return nc


def rope_tables():
    inv_freq = 1.0 / (ROPE_BASE ** (np.arange(0, D, 2, dtype=np.float64) / D))
    t = np.arange(T, dtype=np.float64)
    freqs = np.outer(t, inv_freq)                      # [T, 32]
    cosT = np.cos(freqs).T.astype(np.float32)          # [32, T]
    sinT = np.sin(freqs).T.astype(np.float32)
    cos4 = np.tile(cosT, (4, 1))                       # [128, T]
    sin4 = np.tile(sinT, (4, 1))
    return np.ascontiguousarray(np.stack([cos4, sin4], axis=1))  # [128,2,T]


def perm_matrix():
    pm = np.zeros((P, P), dtype=np.float32)
    for base in (0, 64):
        for d in range(32):
            pm[base + d + 32, base + d] = -1.0       # rot_half: -x2 into top
            pm[base + d, base + d + 32] = 1.0        # +x1 into bottom
    return pm


def host_inputs(x, W_qkv, b_qkv, W_proj, b_proj):
    import ml_dtypes
    bf16 = ml_dtypes.bfloat16
    x = np.asarray(x, dtype=np.float32)
    W_qkv = np.asarray(W_qkv, dtype=np.float32)
    b_qkv = np.asarray(b_qkv, dtype=np.float32)
    W_proj = np.asarray(W_proj, dtype=np.float32)
    scale = 1.0 / math.sqrt(D)
    cs = rope_tables().astype(bf16)
    tri = np.zeros((P, P), dtype=np.float32)
    for c_ in range(P):
        tri[c_, c_ + 1:] = -64.0
    tri = np.ascontiguousarray(tri.astype(bf16))
    pm = np.ascontiguousarray(perm_matrix().astype(bf16))

    in_maps = []
    for core in range(NCORES):
        b = core // 2
        hg = core % 2
        s = hg * DL
        # wqk: [j, p, cc, n]; j = 2g + kind; psq row n = h2*64 + d;
        # W col = kind*C + s + (2g + h2)*64 + d ; input channel = cc*128+p
        cols = np.empty((8, P), dtype=np.int64)
        for j in range(8):
            g, kind = j // 2, j % 2
            for n in range(P):
                h2, d = n // 64, n % 64
                cols[j, n] = kind * C + s + (2 * g + h2) * 64 + d
        wqk_d = np.empty((8, P, 8, P), dtype=np.float32)
        for j in range(8):
            wcols = W_qkv[:, cols[j]]                    # [1024, 128]
            if j % 2 == 0:                               # Q: fold 1/sqrt(D)
                wcols = wcols * scale
            wqk_d[j] = wcols.reshape(8, P, P).transpose(1, 0, 2)
        wqk_d = np.ascontiguousarray(wqk_d.astype(bf16))
        wv_f = W_qkv[:, 2 * C + s:2 * C + s + DL]        # [1024, 512]
        wv_d = np.ascontiguousarray(
            wv_f.reshape(8, P, DL).transpose(1, 0, 2).astype(bf16))
        # wp rows: e2-row p of g-tile = head 2g + p//64, e = p%64
        wp_d = np.empty((P, 4, C), dtype=np.float32)
        for g in range(4):
            for p_ in range(P):
                head = 2 * g + (p_ // 64)
                wp_d[p_, g] = W_proj[s + head * 64 + (p_ % 64), :]
        wp_d = np.ascontiguousarray(wp_d.astype(bf16))
        bias_d = np.zeros((P, 8 + DL), dtype=np.float32)
        for j in range(8):
            bias_d[:, j] = b_qkv[cols[j]]
            if j % 2 == 0:
                bias_d[:, j] *= scale
        bias_d[:, 8:] = np.tile(b_qkv[2 * C + s:2 * C + s + DL][None, :],
                                (P, 1))
        in_maps.append({
            "xb": np.ascontiguousarray(x[b].astype(bf16)),
            "wqk": wqk_d, "wv": wv_d, "wp": wp_d,
            "cs": cs, "bias": np.ascontiguousarray(bias_d), "tri": tri,
            "perm": pm,
        })
    return in_maps


_NC_CACHE = {}


def run(in_maps, **kwargs):
    if "nc" not in _NC_CACHE:
        _NC_CACHE["nc"] = build_nc()
    return run_bass_kernel_spmd(
        _NC_CACHE["nc"], in_maps, core_ids=list(range(NCORES)), **kwargs)


def kernel(x, W_qkv, b_qkv, W_proj, b_proj, **extra):
    in_maps = host_inputs(x, W_qkv, b_qkv, W_proj, b_proj)
    res = run(in_maps)
    b_proj = np.asarray(b_proj, dtype=np.float32)
    out = np.empty((B, T, C), dtype=np.float32)
    for b in range(B):
        out[b] = (res.results[2 * b]["out"].astype(np.float32)
                  + res.results[2 * b + 1]["out"].astype(np.float32) + b_proj)
    return out


# revision 60
# speedup vs baseline: 1.2288x; 1.0203x over previous
"""Trainium2 Bass kernel for multi-head causal attention with RoPE.

Problem: x[4,2048,1024] -> MHA(16 heads, head_dim 64, RoPE, causal) -> [4,2048,1024]

Sharding: 8 cores = 4 batches x 2 head-groups (8 heads each, Megatron-style).
Each core computes a partial [T, C] projection output for its batch; the host
sums the two head-group partials per batch and adds b_proj.

Per-core dataflow, chunked by 512-row t-blocks:
  A(tcn): x^T via DMA-engine xbar transposes (HBM -> SBUF, zero PE/DVE cost),
          Q^T/K^T bf16 GEMMs into a [2 heads x 64d] row layout with RoPE via
          a rot-half permutation matmul + elementwise muls; V in [t, h*64+e]
          bf16 with the qkv bias fused into the PSUM->SBUF copy
  B(qc=tcn): per head-pair (2g, 2g+1): scores S^T = K Q^T (bf16, K=64);
          causal masking of diagonal blocks by a -64 triangular matmul
          accumulated into the score PSUM pre-exp (no vector mask work);
          exp on ACT to bf16; PV flipped (P^T stationary, V moving) so all
          128 output partitions are useful y[q, e]; softmax denominator via
          a ones-column matmul; accumulation groups share a PSUM bank, so
          banks are pre-zeroed and all PV matmuls run with start=False
          (a start=True would mark the whole 2KB bank pending-zero and
          clobber sibling groups); normalize = one reciprocal + broadcast
          multiply per head pair; y -> y^T via one SBUF xbar-transpose DMA
          per chunk
  C(tcn): y^T @ W_proj (bf16), one batched out DMA per chunk

DMA instruction count is minimized (19 total): in this cost model each HWDGE
DMA serializes globally at ~3us (dge delay + transfer + sem propagation), so
per-chunk batching of the x-transpose, y-transpose and output store matters
more than transfer size. Weight loads are ordered by first use so the PE can
start ~6us in; chunk-0's slot-1 QK weights ride the b_phase(0) feed.
"""

import math
import sys

import numpy as np

if "/opt/trn_rl_repo" not in sys.path:
    sys.path.insert(0, "/opt/trn_rl_repo")

import concourse.bass as bass
import concourse.tile as tile
from concourse import bacc
from concourse import mybir
from concourse.bass_utils import run_bass_kernel_spmd
from concourse.masks import make_identity

B, T, C = 4, 2048, 1024
NH, D = 16, 64
HL = 8              # local heads per core
DL = HL * D         # 512
NCORES = 8
P = 128
TCH = 512           # t-chunk width
NTC = T // TCH
ROPE_BASE = 10000.0

F32 = mybir.dt.float32
BF16 = mybir.dt.bfloat16
Exp = mybir.ActivationFunctionType.Exp
Mul = mybir.AluOpType.mult
Add = mybir.AluOpType.add


def _emit(tc, xb, wqk, wv, wp, cs, bias, tri, perm, out, dbg=None):
    nc = tc.nc
    with tc.tile_pool(name="pers", bufs=1) as pers:
        wqk_sb = pers.tile([P, 8, 8, P], BF16)    # [p, j, cc, n]
        wv_sb = pers.tile([P, 8, DL], BF16)       # [p, cc, h*64+e]
        wp_sb = pers.tile([P, 4, C], BF16)        # [e2, g, n]
        cs_sb = pers.tile([P, 2, T], BF16)        # cos/sin, row r -> freq r%32
        bias_sb = pers.tile([P, 8 + DL], F32)
        tri_sb = pers.tile([P, P], BF16)          # tri[c,k] = -64*[c<k]
        perm_sb = pers.tile([P, P], BF16)         # rot-half permutation
        ident = pers.tile([P, P], BF16)
        make_identity(nc, ident)
        oneb = pers.tile([P, 1], BF16)
        nc.vector.memset(oneb[:], 1.0)
        # Q^T/K^T bf16: [row = h2*64 + d, j = 2g + kind (Q/K of pair g), t]
        qkb = pers.tile([P, 8, T], BF16)
        # V bf16: [t%128, t-tile, head, e | ones]; col 64 feeds the
        # softmax denominator through the same PV matmul
        vsb = pers.tile([P, 16, HL, D + 1], BF16)
        nc.vector.memset(vsb.rearrange("p a b c -> p (a b) c")[:, :, D:], 1.0)
        bias_v = bias_sb[:, 8:].rearrange("p (h e) -> p h e", e=D)

        # first-use-ordered weight loads; xT chunk-0 transpose interleaves
        nc.scalar.dma_start(wqk_sb[:, 0:2],
                            wqk[0:2].rearrange("j p cc n -> p j cc n"))

        with tc.tile_pool(name="xT", bufs=3) as pxT, \
             tc.tile_pool(name="t1", bufs=3) as pt1, \
             tc.tile_pool(name="tmp", bufs=4) as ptmp, \
             tc.tile_pool(name="pt", bufs=8) as ppt, \
             tc.tile_pool(name="y", bufs=2) as py, \
             tc.tile_pool(name="rcp", bufs=2) as prcp, \
             tc.tile_pool(name="yT", bufs=2) as pyT, \
             tc.tile_pool(name="ost", bufs=2) as post, \
             tc.tile_pool(name="psS", bufs=2, space="PSUM") as psS, \
             tc.tile_pool(name="psQ", bufs=2, space="PSUM") as psQ, \
             tc.tile_pool(name="psO", bufs=2, space="PSUM") as psO_p:

            def a_units(tcn):
                """Chunk tcn's QKV phase: xbar-transpose DMA, QK GEMM+RoPE,
                V GEMM. Units sized ~0.9us for fine interleaving."""
                ts0 = tcn * TCH
                xT = pxT.tile([P, 8, TCH], BF16, tag="xT")
                units = []

                def xt_unit(xT=xT, ts0=ts0):
                    nc.sync.dma_start_transpose(
                        xT[:], xb[ts0: ts0 + TCH, :])
                xt_list = [xt_unit]

                qk_st = {}

                def qk_half_a(j, xT=xT):
                    psq = psQ.tile([P, TCH], F32, tag="q")
                    qk_st[j] = psq
                    for cc in range(4):
                        nc.tensor.matmul(
                            psq[:],
                            wqk_sb[:, j, cc, :],
                            xT[:, cc, :],
                            start=(cc == 0), stop=False)

                def qk_unit(j, xT=xT, ts0=ts0):
                    psq = qk_st.pop(j)
                    for cc in range(4, 8):
                        nc.tensor.matmul(
                            psq[:],
                            wqk_sb[:, j, cc, :],
                            xT[:, cc, :],
                            start=False, stop=(cc == 7))
                    t1 = pt1.tile([P, TCH], BF16, tag="t1")
                    nc.vector.tensor_scalar_add(t1[:], psq[:],
                                                bias_sb[:, j:j + 1])
                    # psq is dead after the bias copy; reuse its bank for
                    # the rot-half permutation product (keeps psQ at one
                    # allocation per unit so the 2-buf ring never wraps
                    # onto a live tile)
                    nc.tensor.matmul(psq[:], perm_sb[:], t1[:],
                                     start=True, stop=True,
                                     skip_group_check=True)
                    dst = qkb[:, j, ts0:ts0 + TCH]
                    nc.vector.tensor_tensor(dst, t1[:],
                                            cs_sb[:, 0, ts0:ts0 + TCH], Mul)
                    swp = ptmp.tile([P, TCH], BF16, tag="tmp")
                    nc.vector.tensor_tensor(swp[:], psq[:],
                                            cs_sb[:, 1, ts0:ts0 + TCH], Mul)
                    nc.vector.tensor_tensor(dst, dst, swp[:], Add)
                for j in range(4):
                    units.append(lambda j=j: qk_half_a(j))
                    units.append(lambda j=j: qk_unit(j))

                def v_half_a(i, xT=xT):
                    psv = psQ.tile([P, DL], F32, tag="q")
                    qk_st[8 + i] = psv
                    for cc in range(4):
                        nc.tensor.matmul(
                            psv[:],
                            xT[:, cc, i * P:(i + 1) * P],
                            wv_sb[:, cc, :],
                            start=(cc == 0), stop=False)

                def v_unit(i, xT=xT, tcn=tcn):
                    ti = tcn * (TCH // P) + i
                    psv = qk_st.pop(8 + i)
                    for cc in range(4, 8):
                        nc.tensor.matmul(
                            psv[:],
                            xT[:, cc, i * P:(i + 1) * P],
                            wv_sb[:, cc, :],
                            start=False, stop=(cc == 7))
                    psvh = psv.rearrange("p (h e) -> p h e", e=D)
                    nc.vector.tensor_tensor(vsb[:, ti, :, 0:D], psvh,
                                            bias_v, Add)
                for i in range(TCH // P):
                    units.append(lambda i=i: v_half_a(i))
                    units.append(lambda i=i: v_unit(i))
                for j in range(4, 8):
                    units.append(lambda j=j: qk_half_a(j))
                    units.append(lambda j=j: qk_unit(j))
                return xt_list, units

            def c_units(tcn, yTt):
                """Projection for chunk tcn; needs yTt complete."""
                ts0 = tcn * TCH
                units = []

                ost = post.tile([P, 4, C], BF16, tag="ost",
                                name=f"ost_{tcn}")

                def c_unit(i, n, yTt=yTt, ost=ost):
                    psp = psQ.tile([P, 512], F32, tag="q")
                    for g in range(4):
                        nc.tensor.matmul(
                            psp[:],
                            yTt[:, g, i * P:(i + 1) * P],
                            wp_sb[:, g, n * 512:(n + 1) * 512],
                            start=(g == 0), stop=(g == 3))
                    nc.vector.tensor_copy(
                        ost[:, i, n * 512:(n + 1) * 512], psp[:])

                def c_flush_half(h, ost=ost, ts0=ts0):
                    nc.sync.dma_start(
                        out[ts0 + h * 256: ts0 + (h + 1) * 256, :].rearrange(
                            "(i p) c -> p i c", p=P), ost[:, 2 * h:2 * h + 2])

                def c_flush(ost=ost, ts0=ts0):
                    nc.sync.dma_start(
                        out[ts0: ts0 + TCH, :].rearrange(
                            "(i p) c -> p i c", p=P), ost[:])
                last = (tcn == NTC - 1)
                for i in range(TCH // P):
                    for n in range(2):
                        units.append(lambda i=i, n=n: c_unit(i, n))
                        if last and i == 1 and n == 1:
                            units.append(lambda: c_flush_half(0))
                if last:
                    units.append(lambda: c_flush_half(1))
                else:
                    units.append(c_flush)
                return units

            def b_phase(qc, feed, pre):
                """Attention for q-chunk qc; drains `feed` units into PE
                slack while ACT exps. `pre` = next chunk's x-transpose
                (urgent, wait-free) then the previous chunk's y->yT
                transpose."""
                for f in pre:
                    f()
                nblk = 4 * qc + 4
                yTt = pyT.tile([P, 4, TCH], BF16, tag="yT")
                ych = py.tile([P, 4, 4, P], BF16, tag="y",
                              name=f"ych_{qc}")
                drain = {"done": 0, "seen": 0, "n0": max(1, len(feed))}
                total_kc = 4 * nblk

                def drain_tick():
                    # spread the feed evenly over the phase's kc steps
                    drain["seen"] += 1
                    want = drain["seen"] * drain["n0"] // total_kc
                    while feed and drain["done"] < want:
                        feed.pop(0)()
                        drain["done"] += 1
                for g in range(4):
                    psO0 = psO_p.tile([P, 4, D + 1], F32, tag="o",
                                      name=f"psO0_{qc}_{g}")
                    psO1 = psO_p.tile([P, 4, D + 1], F32, tag="o",
                                      name=f"psO1_{qc}_{g}")
                    psO = (psO0, psO1)
                    # pre-zero: 4 accumulation groups share each bank; a
                    # start=True would mark the whole 2KB bank pending-zero
                    # and clobber sibling groups, so accumulate-only.
                    nc.vector.memset(psO0[:], 0.0)
                    nc.vector.memset(psO1[:], 0.0)
                    pv_q = []          # (kc, pt) with 3-block lag

                    def pv_blk(kc, pt, qc=qc, g=g, psO=psO):
                        for hh in range(2):
                            head = 2 * g + hh
                            for qi in range(4):
                                last_kc = 4 * qc + qi
                                if kc > last_kc:
                                    continue
                                stop = (kc == last_kc)
                                lhs = pt[:, hh * 512 + qi * P:
                                         hh * 512 + (qi + 1) * P]
                                nc.tensor.matmul(
                                    psO[hh][:, qi, :], lhs,
                                    vsb[:, kc, head, :],
                                    start=False, stop=stop,
                                    skip_group_check=True)

                    for kc in range(nblk):
                        if qc == 0 and feed and kc < 2:
                            feed.pop(0)()     # chunk-0 carry must emit early
                        elif kc >= 2:
                            drain_tick()
                        m = kc - 4 * qc
                        pt = ppt.tile([P, 1024], BF16, tag="pt",
                                      name=f"pt_{qc}_{g}_{kc}")
                        pss = psS.tile([P, 1024], F32, tag="s")
                        for hh in range(2):
                            q0 = m * P if m > 0 else 0
                            nc.tensor.matmul(
                                pss[:, hh * 512 + q0:(hh + 1) * 512],
                                qkb[64 * hh:64 * hh + 64, 2 * g + 1,
                                    kc * P:(kc + 1) * P],
                                qkb[64 * hh:64 * hh + 64, 2 * g,
                                    qc * TCH + q0:(qc + 1) * TCH],
                                start=True, stop=(m < 0),
                                skip_group_check=True,
                                tile_position=(64 * hh, 0))
                            if m >= 0:
                                nc.tensor.matmul(
                                    pss[:, hh * 512 + q0:
                                        hh * 512 + q0 + P],
                                    tri_sb[:], ident[:],
                                    start=False, stop=True,
                                    skip_group_check=True)
                        with nc.allow_low_precision(reason="bf16 softmax"):
                            if m < 0:
                                nc.scalar.activation(pt[:], pss[:], Exp)
                            else:
                                q0 = m * P
                                nc.scalar.activation(
                                    pt.rearrange("p (h w) -> p h w",
                                                 h=2)[:, :, q0:],
                                    pss.rearrange("p (h w) -> p h w",
                                                  h=2)[:, :, q0:],
                                    Exp)
                        if len(pv_q) >= 4:
                            pv_blk(*pv_q.pop(0))
                        pv_q.append((kc, pt))
                    while pv_q:
                        pv_blk(*pv_q.pop(0))

                    rcp = prcp.tile([P, 2, 4], F32, tag="rcp")
                    yv = ych[:, g].rearrange("p a (h e) -> p h a e", h=2)
                    for hh in range(2):
                        nc.vector.reciprocal(rcp[:, hh],
                                             psO[hh][:, :, D])
                        nc.vector.tensor_tensor(
                            yv[:, hh], psO[hh][:, :, 0:D],
                            rcp[:, hh].rearrange(
                                "p (a o) -> p a o", o=1).to_broadcast(
                                    (P, 4, D)),
                            Mul)
                    if feed and g < 3:
                        feed.pop(0)()
                    if qc == NTC - 1 and g == 1:
                        # last chunk: transpose the first half early so the
                        # final projection's g0/g1 matmuls start immediately
                        nc.sync.dma_start_transpose(
                            yTt[:, 0:2].rearrange(
                                "p g (a c) -> p (g a) c", c=P),
                            ych[:, 0:2])
                while feed:
                    feed.pop(0)()

                if qc == NTC - 1:
                    def ytr(ych=ych, yTt=yTt):
                        nc.sync.dma_start_transpose(
                            yTt[:, 2:4].rearrange(
                                "p g (a c) -> p (g a) c", c=P),
                            ych[:, 2:4])
                else:
                    def ytr(ych=ych, yTt=yTt):
                        nc.sync.dma_start_transpose(
                            yTt.rearrange("p g (a c) -> p (g a) c", c=P),
                            ych[:])
                return yTt, [ytr]

            xt0, a0 = a_units(0)
            nc.scalar.dma_start(bias_sb[:], bias)
            nc.scalar.dma_start(tri_sb[:], tri)
            nc.scalar.dma_start(perm_sb[:], perm)
            for u in xt0:              # x^T chunk-0 transpose (SP queue)
                u()
            nc.scalar.dma_start(cs_sb[:], cs)
            nc.scalar.dma_start(wqk_sb[:, 2:4],
                                wqk[2:4].rearrange("j p cc n -> p j cc n"))
            for u in a0[:8]:           # qk of head-pairs 0,1 (j=0..3)
                u()
            nc.sync.dma_start(wv_sb[:], wv)
            nc.scalar.dma_start(wqk_sb[:, 4:8],
                                wqk[4:8].rearrange("j p cc n -> p j cc n"))
            nc.sync.dma_start(wp_sb[:], wp)
            for u in a0[8:16]:         # v units (needed by b_phase(0) PV)
                u()
            # chunk-0 QK of head-pairs 2,3 ride as b_phase(0) feed: their
            # weights (wqk[4:8]) land late in the DMA chain and must not
            # block the first head-pairs' scores in PE program order; they
            # drain before g=2 needs them
            yT_prev, ytr_prev = None, []
            carry = a0[16:]
            for tcn in range(NTC):
                feed = list(carry)
                carry = []
                pre = []
                if yT_prev is not None:
                    feed.extend(c_units(tcn - 1, yT_prev))
                if tcn + 1 < NTC:
                    xt_n, a_n = a_units(tcn + 1)
                    pre.extend(xt_n)
                    feed.extend(a_n)
                pre.extend(ytr_prev)
                yT_prev, ytr_prev = b_phase(tcn, feed, pre)
            for f in ytr_prev:
                f()
            for u in c_units(NTC - 1, yT_prev):
                u()
            if dbg is not None:
                nc.sync.dma_start(dbg["qkb"], qkb[:])
                nc.sync.dma_start(dbg["vsb"], vsb[:])
                nc.sync.dma_start(dbg["yT3"], yT_prev[:])


def build_nc(debug=False):
    nc = bacc.Bacc("TRN2", target_bir_lowering=False, debug=False)
    xb = nc.dram_tensor("xb", [T, C], BF16, kind="ExternalInput").ap()
    wqk = nc.dram_tensor("wqk", [8, P, 8, P], BF16, kind="ExternalInput").ap()
    wv = nc.dram_tensor("wv", [P, 8, DL], BF16, kind="ExternalInput").ap()
    wp = nc.dram_tensor("wp", [P, 4, C], BF16, kind="ExternalInput").ap()
    cs = nc.dram_tensor("cs", [P, 2, T], BF16, kind="ExternalInput").ap()
    bias = nc.dram_tensor("bias", [P, 8 + DL], F32, kind="ExternalInput").ap()
    tri = nc.dram_tensor("tri", [P, P], BF16, kind="ExternalInput").ap()
    perm = nc.dram_tensor("perm", [P, P], BF16, kind="ExternalInput").ap()
    out = nc.dram_tensor("out", [T, C], BF16, kind="ExternalOutput").ap()
    dbg = None
    if debug:
        dbg = {
            "qkb": nc.dram_tensor("d_qkb", [P, 8, T], BF16,
                                  kind="ExternalOutput").ap(),
            "vsb": nc.dram_tensor("d_vsb", [P, 16, HL, D], BF16,
                                  kind="ExternalOutput").ap(),
            "yT3": nc.dram_tensor("d_yT3", [P, 4, TCH], BF16,
                                  kind="ExternalOutput").ap(),
        }
    with tile.TileContext(nc) as tc:
        _emit(tc, xb, wqk, wv, wp, cs, bias, tri, perm, out, dbg=dbg)
    nc.compile()
    return nc


def rope_tables():
    inv_freq = 1.0 / (ROPE_BASE ** (np.arange(0, D, 2, dtype=np.float64) / D))
    t = np.arange(T, dtype=np.float64)
    freqs = np.outer(t, inv_freq)                      # [T, 32]
    cosT = np.cos(freqs).T.astype(np.float32)          # [32, T]
    sinT = np.sin(freqs).T.astype(np.float32)
    cos4 = np.tile(cosT, (4, 1))                       # [128, T]
    sin4 = np.tile(sinT, (4, 1))
    return np.ascontiguousarray(np.stack([cos4, sin4], axis=1))  # [128,2,T]


def perm_matrix():
    pm = np.zeros((P, P), dtype=np.float32)
    for base in (0, 64):
        for d in range(32):
            pm[base + d + 32, base + d] = -1.0       # rot_half: -x2 into top
            pm[base + d, base + d + 32] = 1.0        # +x1 into bottom
    return pm


def host_inputs(x, W_qkv, b_qkv, W_proj, b_proj):
    import ml_dtypes
    bf16 = ml_dtypes.bfloat16
    x = np.asarray(x, dtype=np.float32)
    W_qkv = np.asarray(W_qkv, dtype=np.float32)
    b_qkv = np.asarray(b_qkv, dtype=np.float32)
    W_proj = np.asarray(W_proj, dtype=np.float32)
    scale = 1.0 / math.sqrt(D)
    cs = rope_tables().astype(bf16)
    tri = np.zeros((P, P), dtype=np.float32)
    for c_ in range(P):
        tri[c_, c_ + 1:] = -64.0
    tri = np.ascontiguousarray(tri.astype(bf16))
    pm = np.ascontiguousarray(perm_matrix().astype(bf16))

    in_maps = []
    for core in range(NCORES):
        b = core // 2
        hg = core % 2
        s = hg * DL
        # wqk: [j, p, cc, n]; j = 2g + kind; psq row n = h2*64 + d;
        # W col = kind*C + s + (2g + h2)*64 + d ; input channel = cc*128+p
        cols = np.empty((8, P), dtype=np.int64)
        for j in range(8):
            g, kind = j // 2, j % 2
            for n in range(P):
                h2, d = n // 64, n % 64
                cols[j, n] = kind * C + s + (2 * g + h2) * 64 + d
        wqk_d = np.empty((8, P, 8, P), dtype=np.float32)
        for j in range(8):
            wcols = W_qkv[:, cols[j]]                    # [1024, 128]
            if j % 2 == 0:                               # Q: fold 1/sqrt(D)
                wcols = wcols * scale
            wqk_d[j] = wcols.reshape(8, P, P).transpose(1, 0, 2)
        wqk_d = np.ascontiguousarray(wqk_d.astype(bf16))
        wv_f = W_qkv[:, 2 * C + s:2 * C + s + DL]        # [1024, 512]
        wv_d = np.ascontiguousarray(
            wv_f.reshape(8, P, DL).transpose(1, 0, 2).astype(bf16))
        # wp rows: e2-row p of g-tile = head 2g + p//64, e = p%64
        wp_d = np.empty((P, 4, C), dtype=np.float32)
        for g in range(4):
            for p_ in range(P):
                head = 2 * g + (p_ // 64)
                wp_d[p_, g] = W_proj[s + head * 64 + (p_ % 64), :]
        wp_d = np.ascontiguousarray(wp_d.astype(bf16))
        bias_d = np.zeros((P, 8 + DL), dtype=np.float32)
        for j in range(8):
            bias_d[:, j] = b_qkv[cols[j]]
            if j % 2 == 0:
                bias_d[:, j] *= scale
        bias_d[:, 8:] = np.tile(b_qkv[2 * C + s:2 * C + s + DL][None, :],
                                (P, 1))
        in_maps.append({
            "xb": np.ascontiguousarray(x[b].astype(bf16)),
            "wqk": wqk_d, "wv": wv_d, "wp": wp_d,
            "cs": cs, "bias": np.ascontiguousarray(bias_d), "tri": tri,
            "perm": pm,
        })
    return in_maps


_NC_CACHE = {}


def run(in_maps, **kwargs):
    if "nc" not in _NC_CACHE:
        _NC_CACHE["nc"] = build_nc()
    return run_bass_kernel_spmd(
        _NC_CACHE["nc"], in_maps, core_ids=list(range(NCORES)), **kwargs)


def kernel(x, W_qkv, b_qkv, W_proj, b_proj, **extra):
    in_maps = host_inputs(x, W_qkv, b_qkv, W_proj, b_proj)
    res = run(in_maps)
    b_proj = np.asarray(b_proj, dtype=np.float32)
    out = np.empty((B, T, C), dtype=np.float32)
    for b in range(B):
        out[b] = (res.results[2 * b]["out"].astype(np.float32)
                  + res.results[2 * b + 1]["out"].astype(np.float32) + b_proj)
    return out
